# revision 26
# baseline (speedup 1.0000x reference)
"""nn_Head single-head causal attention on 8 TRN2 NeuronCores.

Full inputs: x [8, 2048, 1024] f32, Wk/Wq/Wv [1024, 64] f32.
Full output: [8, 2048, 64] f32 = softmax(causal(q k^T * C^-0.5)) @ v per batch.

Sharding: data-parallel over batch B=8 -> one batch element per core;
weights replicated. No collectives.

Per-core kernel (Bass/Tile, f32r matmuls + bf16 probability/value stage):
  A) x t-tiles DMA in (first two tiles split in halves to cut first-arrival
     latency); PE-transpose to xT [c-part, t] (TensorE contracts over the
     partition dim, fp32 cannot DMA-transpose).  All PE transposes use a
     bf16 identity as the moving operand: the moving dtype sets the
     cycles/row (bf16 1.0 vs f32r 1.5 / f32 2.0), while values stay exact.
  B) QKV per 512-col block: kT/qT/vT [h(64), t] via lhsT=W [c,64], rhs=xT;
     v transposed back to natural v1 [s-part, t-tile, H+1] bf16 with a ones
     column at H that makes the PV matmul also produce the softmax
     denominator.  W is DMAed in half-tensors interleaved between x tiles.
  C) attention per 256-col block: S^T tile = kT_slice^T@qT -> PSUM
     [s 128, t 256]; exp on ScalarE with scale=C^-0.5 folded in (scores are
     O(1): no max-subtraction needed, mathematically identical softmax);
     causality via memset of fully-masked column ranges + a 0/1
     upper-triangular mask mul on diagonal tiles; PV: po[h|denom, t] +=
     v1_slice^T @ P^T accumulated over s-tiles; po -> SBUF (f32r),
     PE-transpose to [t-part, H+1], multiply by per-partition reciprocal of
     the denominator column, streaming output DMAs.
  Emission interleaves B/C blocks between A tiles in data-arrival order so
  the PE always has ready work (each engine has a 4-deep scoreboard that
  lets ready instructions pass blocked ones).
"""

from contextlib import ExitStack

import numpy as np

import concourse.bass as bass
import concourse.mybir as mybir
import concourse.tile as tile
from concourse import bass_utils
from concourse.masks import make_identity

B, T, C, H = 8, 2048, 1024, 64
N_CORES = 8
P = 128

# ---- tuning knobs (swept via TimelineSim) ----
IDENT_B = True  # bf16 identity for all PE transposes
ACT_CLEAN = True  # keep ScalarE free for exp: xT/kT copies on Pool/DVE
PSA1_PSS3 = False  # psA bufs 1 / psS bufs 3 (vs 2/2)
W_EARLY = False  # W DMA halves at tt1/2/3 (vs tt3/4/5/6)
OUT_TILE = True  # per-tile output DMAs (vs per-block)
ZIP67 = False  # zip the last two attention blocks pair-by-pair
BLKB_W = 512  # qkv block width
SCHED = "v2"  # "v2" ladder or "weave"
PV_OFFSET = 2  # emit PV pairs this many pairs behind S pairs
DIAG_FIRST = False  # emit the diagonal pair first within each C block
PSB1_PSS3 = False  # psB bufs 1, psS bufs 3
LADDER = 9  # emission order variant
DIAG_LAST7 = True  # diagonal pair first in the final block
PT_BUFS = 12  # ptile pool depth
GRP_OFF = 3  # PV offset within a grouped C emission
XT_RATIO = 0  # xT copy engine mix: 0=3:1 DVE:ACT, 1=1:1, 2=all DVE
W_SLOT = 0  # which x-tile slots carry the W DMA halves
PE_POOL_LAST = False  # late C blocks take pe tiles from the idle psA pool


def _patch_drain_split():
    """This walrus build accepts only one sem wait per instruction ("Too many
    sync wait commands" in setupSyncWait otherwise). Hoist extra waits onto
    same-engine NOPs ahead of the instruction (engine streams dispatch
    in-order, so the blocking semantics are identical), and split the
    TileContext tail drain the same way."""
    if getattr(tile.TileContext, "_drain_split_patched", False):
        return
    from concourse.tile import ScopedClock

    _orig_add = tile.TileContext._add_instruction

    def _patched_add(self, inst):
        si = getattr(inst, "sync_info", None)
        if si is not None and si.on_wait and len(si.on_wait) > 1:
            waits = list(si.on_wait)
            for i, w in enumerate(waits[:-1]):
                nop = mybir.InstNoOp(
                    name=f"{inst.name}-ws{i}",
                    sync_info=mybir.SyncInfo(on_wait=[w], on_update=[]),
                    bass_nofuse=True,
                    engine=inst.engine,
                )
                _orig_add(self, nop)
            si.on_wait = waits[-1:]
            inst.sync_info = si
        _orig_add(self, inst)

    tile.TileContext._add_instruction = _patched_add

    def _patched_dab(self, tick_clock, wait_clock):
        nc = self.nc
        drain_inst = nc.sync.drain()
        wait_clock.add_sem_waits(
            drain_inst.ins, ScopedClock({None: tick_clock.global_clock})
        )
        si = drain_inst.ins.sync_info
        if si is not None and si.on_wait and len(si.on_wait) > 1:
            waits = list(si.on_wait)
            si.on_wait = waits[:1]
            drain_inst.ins.sync_info = si
            for w in waits[1:]:
                d2 = nc.sync.drain()
                d2.ins.sync_info = mybir.SyncInfo(on_wait=[w], on_update=[])
        nc.all_engine_barrier()
        popped = nc._tile_sem_poison_stack.pop()
        assert popped is self._sem_poison
        nc.clear_and_free_semaphores(list(self.sems.allocated().values()))
        nc.all_engine_barrier()

    tile.TileContext._drain_and_barrier = _patched_dab
    tile.TileContext._drain_split_patched = True


def _emit(tc, out_d, x_d, wk_d, wq_d, wv_d):
    nc = tc.nc
    f32r = mybir.dt.float32r
    f32 = mybir.dt.float32
    bf16 = mybir.dt.bfloat16
    Exp = mybir.ActivationFunctionType.Exp

    CT = C // P  # 8 c-tiles
    TT = T // P  # 16 t-tiles
    BLKB = BLKB_W  # qkv block width
    BLK = 256  # attention block width
    H1 = H + 1
    scale = float(C) ** -0.5
    tdt = bf16  # vT/oT dtype: bf16 so their transposes use the bf16 identity

    with ExitStack() as ctx:
        const = ctx.enter_context(tc.tile_pool(name="const", bufs=1))
        persist = ctx.enter_context(tc.tile_pool(name="persist", bufs=1))
        xa_pool = ctx.enter_context(tc.tile_pool(name="xa", bufs=TT))
        pt_pool = ctx.enter_context(tc.tile_pool(name="ptp", bufs=PT_BUFS))
        oT_pool = ctx.enter_context(tc.tile_pool(name="otp", bufs=2))
        rec_pool = ctx.enter_context(tc.tile_pool(name="recp", bufs=2))
        # PSUM: 8 banks total so all phases can overlap.
        psA = ctx.enter_context(
            tc.tile_pool(name="psA", bufs=1 if PSA1_PSS3 else 2, space="PSUM")
        )
        psB = ctx.enter_context(tc.tile_pool(name="psB", bufs=1 if PSB1_PSS3 else 2, space="PSUM"))
        psS = ctx.enter_context(
            tc.tile_pool(
                name="psS",
                bufs=3 if (PSA1_PSS3 or PSB1_PSS3) else 2,
                space="PSUM",
            )
        )
        psO = ctx.enter_context(tc.tile_pool(name="psO", bufs=2, space="PSUM"))

        # identity: build in f32 (memset on f32r is invalid ISA in this
        # walrus); f32r and bf16 copies for dtype-matched transposes
        ident = const.tile([P, P], f32, name="ident")
        make_identity(nc, ident)
        identr = const.tile([P, P], f32r, name="identr")
        nc.vector.tensor_copy(out=identr, in_=ident)
        identb = const.tile([P, P], bf16, name="identb")
        nc.vector.tensor_copy(out=identb, in_=ident)
        idT = identb if IDENT_B else identr
        # 0/1 mask: mask[s, t] = 1 iff s <= t (keep causal entries)
        mask = const.tile([P, P], bf16, name="mask")
        nc.vector.memset(mask, 1.0)
        nc.gpsimd.affine_select(
            out=mask,
            in_=mask,
            compare_op=mybir.AluOpType.is_ge,
            fill=0.0,
            base=0,
            pattern=[[1, P]],
            channel_multiplier=-1,
        )

        # [Wk | Wq] packed: one M=128 matmul produces k on partitions 0-63
        # and q on 64-127
        wkq_sb = const.tile([P, CT, 2 * H], f32r, name="wkq_sb")
        wv_sb = const.tile([P, CT, H], f32r, name="wv_sb")

        xT = persist.tile([P, CT, T], f32r, name="xT")
        kT = persist.tile([H, T], f32r, name="kT")
        qT = persist.tile([H, T], f32r, name="qT")
        vT = persist.tile([H, T], tdt, name="vT")
        v1 = persist.tile([P, TT, H1], bf16, name="v1")
        out_sb = persist.tile([P, TT, H], f32, name="out_sb")

        nc.vector.memset(v1[:, :, H : H + 1], 1.0)

        # ---- DMA emission (SP stream order == DMA device service order).
        wk_r = wk_d.rearrange("(o p) h -> p o h", p=P)
        wq_r = wq_d.rearrange("(o p) h -> p o h", p=P)
        wv_r = wv_d.rearrange("(o p) h -> p o h", p=P)
        wslots = {
            0: {3: 0, 4: 1, 5: 2, 6: 3},
            1: {2: 0, 3: 1, 4: 2, 5: 3},
            2: {1: 0, 2: 1, 3: 2, 4: 3},
        }[W_SLOT]
        xa_tiles = []
        for tt in range(TT):
            tsl = slice(tt * P, (tt + 1) * P)
            xa = xa_pool.tile([P, C], f32r, name="xa")
            xa_tiles.append(xa)
            if tt < 2:
                nc.sync.dma_start(xa[:, 0 : C // 2], x_d[tsl, 0 : C // 2])
                nc.sync.dma_start(xa[:, C // 2 : C], x_d[tsl, C // 2 : C])
            else:
                nc.sync.dma_start(xa, x_d[tsl, :])
            w = wslots.get(tt)
            if False:
                if w == 0:
                    nc.sync.dma_start(wkq_sb[:, :, 0:H], wk_r)
                elif w == 1:
                    nc.sync.dma_start(wkq_sb[:, :, H : 2 * H], wq_r)
                elif w == 2:
                    nc.sync.dma_start(wv_sb, wv_r)
            elif w is not None:
                if w == 0:
                    nc.sync.dma_start(wkq_sb[:, 0:4, 0:H], wk_r[:, 0:4, :])
                    nc.sync.dma_start(
                        wkq_sb[:, 0:4, H : 2 * H], wq_r[:, 0:4, :]
                    )
                elif w == 1:
                    nc.sync.dma_start(wkq_sb[:, 4:CT, 0:H], wk_r[:, 4:CT, :])
                    nc.sync.dma_start(
                        wkq_sb[:, 4:CT, H : 2 * H], wq_r[:, 4:CT, :]
                    )
                elif w == 2:
                    nc.sync.dma_start(wv_sb[:, 0:4, :], wv_r[:, 0:4, :])
                elif w == 3:
                    nc.sync.dma_start(wv_sb[:, 4:CT, :], wv_r[:, 4:CT, :])

        # ---- compute emission helpers
        copy_rr = [0]
        if XT_RATIO == 0:
            copy_engines = [nc.vector, nc.vector, nc.scalar, nc.vector]
        elif XT_RATIO == 1:
            copy_engines = [nc.vector, nc.scalar]
        else:
            copy_engines = [nc.vector]

        def phase_a(tt):
            """transpose x tile tt into xT."""
            tsl = slice(tt * P, (tt + 1) * P)
            xa = xa_tiles[tt]
            for cg in range(2):
                ps_t = psA.tile([P, 4, P], f32r, name="ps_t", tag="pst")
                for j in range(4):
                    ci = cg * 4 + j
                    nc.tensor.transpose(
                        ps_t[:, j, :], xa[:, ci * P : (ci + 1) * P], identr
                    )
                dst = xT[:, cg * 4 : cg * 4 + 4, tsl]
                eng = copy_engines[copy_rr[0] % len(copy_engines)]
                copy_rr[0] += 1
                if eng is nc.scalar:
                    nc.scalar.copy(out=dst, in_=ps_t)
                else:
                    eng.tensor_copy(out=dst, in_=ps_t)

        def phase_b_kq(c0, c1):
            tsl = slice(c0, c1)
            pkq = psB.tile([P, c1 - c0], f32, name="pkq", tag="bt")
            for ci in range(CT):
                nc.tensor.matmul(
                    pkq,
                    wkq_sb[:, ci, :],
                    xT[:, ci, tsl],
                    start=(ci == 0),
                    stop=(ci == CT - 1),
                )
            nc.scalar.copy(out=kT[:, tsl], in_=pkq[0:H, :])
            # partition-shift copy 64-127 -> 0-63 (legal on DVE)
            nc.vector.tensor_copy(out=qT[:, tsl], in_=pkq[H:P, :])

        def phase_b_v(c0, c1):
            tsl = slice(c0, c1)
            pv = psB.tile([H, c1 - c0], f32, name="pv", tag="bt")
            for ci in range(CT):
                nc.tensor.matmul(
                    pv,
                    wv_sb[:, ci, :],
                    xT[:, ci, tsl],
                    start=(ci == 0),
                    stop=(ci == CT - 1),
                )
            nc.scalar.copy(out=vT[:, tsl], in_=pv)
            for c4 in range((c1 - c0) // P):
                st = c0 // P + c4
                pvt = psB.tile([P, H], tdt, name="pvt", tag="bt")
                nc.tensor.transpose(
                    pvt, vT[:, st * P : (st + 1) * P], idT[:H, :H]
                )
                nc.vector.tensor_copy(out=v1[:, st, 0:H], in_=pvt)

        def c_block(cb, diag_first=False, mask_dve=False):
            pe_pool, pe_tag = (psA, "pst") if (
                PE_POOL_LAST and cb >= 6
            ) else (psO, "po")
            """attention block cb: S-pair(g) / PV-pair(g) / finish closures.
            The diagonal pair (g=cb) is emitted FIRST so its mask chain is
            off the block's critical tail; PSUM start/stop flags follow
            emission order (accumulation order is irrelevant to the sum)."""
            tsl = slice(cb * BLK, (cb + 1) * BLK)
            state = {}
            order = [cb] + list(range(cb)) if diag_first else list(range(cb + 1))
            first_pair = order[0]
            last_pair = order[-1]

            def start():
                state["po"] = psO.tile([H1, BLK], f32, name="po", tag="po")
                state["pt"] = {}

            def s_pair(g):
                ps_s = psS.tile([P, 2, BLK], f32, name="ps_s")
                for j in range(2):
                    st = 2 * g + j
                    nc.tensor.matmul(
                        ps_s[:, j, :],
                        kT[:, st * P : (st + 1) * P],
                        qT[:, tsl],
                        start=True,
                        stop=True,
                    )
                ptile = pt_pool.tile([P, 2, BLK], bf16, name="ptile")
                state["pt"][g] = ptile
                if g < cb:
                    nc.scalar.activation(ptile, ps_s, Exp, scale=scale)
                else:
                    nc.scalar.activation(
                        ptile[:, 0, :], ps_s[:, 0, :], Exp, scale=scale
                    )
                    nc.scalar.activation(
                        ptile[:, 1, P:BLK], ps_s[:, 1, P:BLK], Exp, scale=scale
                    )
                    nc.gpsimd.memset(ptile[:, 1, 0:P], 0.0)
                    meng = nc.vector if mask_dve else nc.gpsimd
                    meng.tensor_mul(ptile[:, 0, 0:P], ptile[:, 0, 0:P], mask)
                    meng.tensor_mul(
                        ptile[:, 1, P:BLK], ptile[:, 1, P:BLK], mask
                    )

            def pv_pair(g):
                po = state["po"]
                ptile = state["pt"].pop(g)
                for j in range(2):
                    st = 2 * g + j
                    nc.tensor.matmul(
                        po,
                        v1[:, st, 0:H1],
                        ptile[:, j, :],
                        start=(g == first_pair and j == 0),
                        stop=(g == last_pair and j == 1),
                    )

            def finish():
                po = state["po"]
                oT = oT_pool.tile([H1, BLK], tdt, name="oT")
                for c4 in range(BLK // P):
                    csl = slice(c4 * P, (c4 + 1) * P)
                    if c4 % 2 == 0:
                        nc.vector.tensor_copy(out=oT[:, csl], in_=po[:, csl])
                    else:
                        nc.scalar.copy(out=oT[:, csl], in_=po[:, csl])
                    st = cb * (BLK // P) + c4
                    pe = pe_pool.tile([P, H1], tdt, name="pe", tag=pe_tag)
                    nc.tensor.transpose(pe, oT[:, csl], idT[:H1, :H1])
                    rec = rec_pool.tile([P, 1], f32, name="rec")
                    nc.vector.reciprocal(rec, pe[:, H:H1])
                    nc.vector.tensor_scalar_mul(
                        out_sb[:, st, :], pe[:, 0:H], rec
                    )
                    nc.sync.dma_start(
                        out_d.rearrange("(o p) h -> p o h", p=P)[:, st, :],
                        out_sb[:, st, :],
                    )

            return start, s_pair, pv_pair, finish, order

        def phase_c_group(cbs, pv_offset, diag_first_last=False,
                          mask_dve=False):
            """emit several C blocks with all their S-pairs zipped ahead of
            the PV-pairs (offset in pairs) so the exp stream saturates ACT."""
            blocks = {}
            seq = []
            for cb in cbs:
                df = diag_first_last and cb == cbs[-1]
                start, s_pair, pv_pair, finish, order = c_block(
                    cb, df, mask_dve
                )
                start()
                blocks[cb] = (s_pair, pv_pair, finish)
                seq.extend((cb, g) for g in order)
            done = 0
            for i, (cb, g) in enumerate(seq):
                blocks[cb][0](g)
                if i >= pv_offset:
                    pcb, pg = seq[i - pv_offset]
                    blocks[pcb][1](pg)
                    done = i - pv_offset + 1
                    if done < len(seq) and seq[done][0] != pcb and (
                        done == 0 or seq[done - 1][0] == pcb
                    ):
                        blocks[pcb][2]()
            for i in range(done, len(seq)):
                pcb, pg = seq[i]
                blocks[pcb][1](pg)
                if i + 1 == len(seq) or seq[i + 1][0] != pcb:
                    blocks[pcb][2]()

        def phase_c(cb, diag_first=False, mask_dve=False):
            start, s_pair, pv_pair, finish, order = c_block(
                cb, diag_first, mask_dve
            )
            start()
            for i, g in enumerate(order):
                s_pair(g)
                if i >= PV_OFFSET:
                    pv_pair(order[i - PV_OFFSET])
            for i in range(max(0, len(order) - PV_OFFSET), len(order)):
                pv_pair(order[i])
            finish()

        def emit(tokens):
            for t in tokens.split():
                if t.startswith("A"):
                    phase_a(int(t[1:]))
                elif t.startswith("K"):
                    c0, c1 = t[1:].split("-")
                    phase_b_kq(int(c0), int(c1))
                elif t.startswith("V"):
                    c0, c1 = t[1:].split("-")
                    phase_b_v(int(c0), int(c1))
                elif t.startswith("C"):
                    cb = int(t[1:])
                    phase_c(cb, diag_first=(cb == 7 and DIAG_LAST7),
                            mask_dve=(cb >= 6))

        if LADDER == 9:
            phase_a(0)
            phase_a(1)
            phase_a(2)
            phase_a(3)
            phase_b_kq(0, 512)
            phase_b_v(0, 512)
            phase_c_group([0, 1], GRP_OFF)
            phase_a(4)
            phase_a(5)
            phase_a(6)
            phase_a(7)
            phase_b_kq(512, 1024)
            phase_b_v(512, 1024)
            phase_c_group([2, 3], GRP_OFF)
            phase_a(8)
            phase_a(9)
            phase_a(10)
            phase_a(11)
            phase_b_kq(1024, 1536)
            phase_b_v(1024, 1536)
            phase_c_group([4, 5], GRP_OFF)
            phase_a(12)
            phase_a(13)
            phase_b_kq(1536, 1792)
            phase_b_v(1536, 1792)
            phase_c(6, mask_dve=True)
            phase_a(14)
            phase_a(15)
            phase_b_kq(1792, 2048)
            phase_b_v(1792, 2048)
            phase_c(7, diag_first=DIAG_LAST7, mask_dve=True)
            return

        if LADDER == 8:
            phase_a(0)
            phase_a(1)
            phase_a(2)
            phase_a(3)
            phase_b_kq(0, 512)
            phase_a(4)
            phase_b_v(0, 512)
            phase_a(5)
            phase_c_group([0, 1], GRP_OFF)
            phase_a(6)
            phase_a(7)
            phase_b_kq(512, 1024)
            phase_a(8)
            phase_b_v(512, 1024)
            phase_a(9)
            phase_c_group([2, 3], GRP_OFF)
            phase_a(10)
            phase_a(11)
            phase_b_kq(1024, 1536)
            phase_a(12)
            phase_b_v(1024, 1536)
            phase_a(13)
            phase_c_group([4, 5], GRP_OFF)
            phase_a(14)
            phase_a(15)
            phase_b_kq(1536, 1792)
            phase_b_v(1536, 1792)
            phase_b_kq(1792, 2048)
            phase_b_v(1792, 2048)
            phase_c_group([6, 7], GRP_OFF, diag_first_last=DIAG_LAST7,
                          mask_dve=True)
            return

        LADDERS = {
            # best-known v2-style interleave with split last B block
            1: "A0 A1 A2 A3 K0-512 A4 V0-512 A5 C0 C1 A6 A7 "
               "K512-1024 A8 V512-1024 A9 C2 A10 C3 A11 "
               "K1024-1536 A12 V1024-1536 A13 C4 A14 C5 A15 "
               "K1536-1792 V1536-1792 C6 K1792-2048 V1792-2048 C7",
            # C-first everywhere
            2: "A0 A1 A2 A3 K0-512 V0-512 C0 C1 A4 A5 A6 A7 "
               "K512-1024 V512-1024 C2 C3 A8 A9 A10 A11 "
               "K1024-1536 V1024-1536 C4 C5 A12 A13 "
               "K1536-1792 V1536-1792 C6 A14 A15 K1792-2048 V1792-2048 C7",
            # hybrid: keep early interleave, pull late C blocks ahead of A14/A15
            3: "A0 A1 A2 A3 K0-512 A4 V0-512 A5 C0 C1 A6 A7 "
               "K512-1024 A8 V512-1024 A9 C2 A10 C3 A11 "
               "K1024-1536 A12 V1024-1536 A13 C4 C5 A14 A15 "
               "K1536-1792 V1536-1792 C6 K1792-2048 V1792-2048 C7",
            # hybrid + C4 before A13
            4: "A0 A1 A2 A3 K0-512 A4 V0-512 A5 C0 C1 A6 A7 "
               "K512-1024 A8 V512-1024 A9 C2 A10 C3 A11 "
               "K1024-1536 A12 V1024-1536 C4 A13 C5 A14 A15 "
               "K1536-1792 V1536-1792 C6 K1792-2048 V1792-2048 C7",
            # L1 + C2/C3 earlier relative to A9-A11
            5: "A0 A1 A2 A3 K0-512 A4 V0-512 A5 C0 C1 A6 A7 "
               "K512-1024 A8 V512-1024 C2 A9 C3 A10 A11 "
               "K1024-1536 A12 V1024-1536 A13 C4 A14 C5 A15 "
               "K1536-1792 V1536-1792 C6 K1792-2048 V1792-2048 C7",
            7: "A0 A1 A2 A3 K0-256 V0-256 C0 K256-512 V256-512 C1 A4 A5 "
               "K512-768 V512-768 C2 A6 A7 K768-1024 V768-1024 C3 A8 A9 "
               "K1024-1280 V1024-1280 C4 A10 A11 K1280-1536 V1280-1536 C5 "
               "A12 A13 K1536-1792 V1536-1792 C6 A14 A15 "
               "K1792-2048 V1792-2048 C7",
            # L5 + L4 combined
            6: "A0 A1 A2 A3 K0-512 A4 V0-512 A5 C0 C1 A6 A7 "
               "K512-1024 A8 V512-1024 C2 A9 C3 A10 A11 "
               "K1024-1536 A12 V1024-1536 C4 A13 C5 A14 A15 "
               "K1536-1792 V1536-1792 C6 K1792-2048 V1792-2048 C7",
        }
        emit(LADDERS[LADDER])


_NC_CACHE = {}


def build_nc():
    if "nc" in _NC_CACHE:
        return _NC_CACHE["nc"]
    _patch_drain_split()
    f32r = mybir.dt.float32r
    f32 = mybir.dt.float32
    nc = bass.Bass(
        "TRN2", target_bir_lowering=False, debug=False, num_devices=N_CORES
    )
    x_d = nc.dram_tensor("x", [T, C], f32r, kind="ExternalInput").ap()
    wk_d = nc.dram_tensor("Wk", [C, H], f32r, kind="ExternalInput").ap()
    wq_d = nc.dram_tensor("Wq", [C, H], f32r, kind="ExternalInput").ap()
    wv_d = nc.dram_tensor("Wv", [C, H], f32r, kind="ExternalInput").ap()
    out_d = nc.dram_tensor("out", [T, H], f32, kind="ExternalOutput").ap()
    with tile.TileContext(nc) as tc:
        _emit(tc, out_d, x_d, wk_d, wq_d, wv_d)
    _NC_CACHE["nc"] = nc
    return nc


def kernel(x, Wk, Wq, Wv, **run_kwargs):
    """Full-input entry point: shard over batch, run on cores 0-7, gather."""
    x = np.ascontiguousarray(np.asarray(x), dtype=np.float32)
    Wk = np.ascontiguousarray(np.asarray(Wk), dtype=np.float32)
    Wq = np.ascontiguousarray(np.asarray(Wq), dtype=np.float32)
    Wv = np.ascontiguousarray(np.asarray(Wv), dtype=np.float32)
    assert x.shape == (B, T, C), x.shape

    nc = build_nc()
    in_maps = [
        {"x": np.ascontiguousarray(x[b]), "Wk": Wk, "Wq": Wq, "Wv": Wv}
        for b in range(B)
    ]
    res = bass_utils.run_bass_kernel_spmd(
        nc, in_maps, core_ids=list(range(N_CORES)), **run_kwargs
    )
    out = np.stack([res.results[b]["out"] for b in range(B)], axis=0)
    if run_kwargs:
        kernel.last_results = res
    return out.astype(np.float32)


# revision 28
# speedup vs baseline: 1.0049x; 1.0049x over previous
"""nn_Head single-head causal attention on 8 TRN2 NeuronCores.

Full inputs: x [8, 2048, 1024] f32, Wk/Wq/Wv [1024, 64] f32.
Full output: [8, 2048, 64] f32 = softmax(causal(q k^T * C^-0.5)) @ v per batch.

Sharding: data-parallel over batch B=8 -> one batch element per core;
weights replicated. No collectives.

Per-core kernel (Bass/Tile, f32r matmuls + bf16 probability/value stage):
  A) x t-tiles DMA in (first two tiles split in halves to cut first-arrival
     latency); PE-transpose to xT [c-part, t] (TensorE contracts over the
     partition dim, fp32 cannot DMA-transpose).  All PE transposes use a
     bf16 identity as the moving operand: the moving dtype sets the
     cycles/row (bf16 1.0 vs f32r 1.5 / f32 2.0), while values stay exact.
  B) QKV per 512-col block: kT/qT/vT [h(64), t] via lhsT=W [c,64], rhs=xT;
     v transposed back to natural v1 [s-part, t-tile, H+1] bf16 with a ones
     column at H that makes the PV matmul also produce the softmax
     denominator.  W is DMAed in half-tensors interleaved between x tiles.
  C) attention per 256-col block: S^T tile = kT_slice^T@qT -> PSUM
     [s 128, t 256]; exp on ScalarE with scale=C^-0.5 folded in (scores are
     O(1): no max-subtraction needed, mathematically identical softmax);
     causality via memset of fully-masked column ranges + a 0/1
     upper-triangular mask mul on diagonal tiles; PV: po[h|denom, t] +=
     v1_slice^T @ P^T accumulated over s-tiles; po -> SBUF (f32r),
     PE-transpose to [t-part, H+1], multiply by per-partition reciprocal of
     the denominator column, streaming output DMAs.
  Emission interleaves B/C blocks between A tiles in data-arrival order so
  the PE always has ready work (each engine has a 4-deep scoreboard that
  lets ready instructions pass blocked ones).
"""

from contextlib import ExitStack

import numpy as np

import concourse.bass as bass
import concourse.mybir as mybir
import concourse.tile as tile
from concourse import bass_utils
from concourse.masks import make_identity

B, T, C, H = 8, 2048, 1024, 64
N_CORES = 8
P = 128

# ---- tuning knobs (swept via TimelineSim) ----
IDENT_B = True  # bf16 identity for all PE transposes
ACT_CLEAN = True  # keep ScalarE free for exp: xT/kT copies on Pool/DVE
PSA1_PSS3 = False  # psA bufs 1 / psS bufs 3 (vs 2/2)
W_EARLY = False  # W DMA halves at tt1/2/3 (vs tt3/4/5/6)
OUT_TILE = True  # per-tile output DMAs (vs per-block)
ZIP67 = False  # zip the last two attention blocks pair-by-pair
BLKB_W = 512  # qkv block width
SCHED = "v2"  # "v2" ladder or "weave"
PV_OFFSET = 2  # emit PV pairs this many pairs behind S pairs
DIAG_FIRST = False  # emit the diagonal pair first within each C block
PSB1_PSS3 = False  # psB bufs 1, psS bufs 3
LADDER = 9  # emission order variant
DIAG_LAST7 = True  # diagonal pair first in the final block
PT_BUFS = 12  # ptile pool depth
GRP_OFF = 3  # PV offset within a grouped C emission
XT_RATIO = 0  # xT copy engine mix: 0=3:1 DVE:ACT, 1=1:1, 2=all DVE
W_SLOT = 0  # which x-tile slots carry the W DMA halves
PE_POOL_LAST = False  # late C blocks take pe tiles from the idle psA pool
X_SPLIT = 16  # how many leading x tiles DMA in half-tiles


def _patch_drain_split():
    """This walrus build accepts only one sem wait per instruction ("Too many
    sync wait commands" in setupSyncWait otherwise). Hoist extra waits onto
    same-engine NOPs ahead of the instruction (engine streams dispatch
    in-order, so the blocking semantics are identical), and split the
    TileContext tail drain the same way."""
    if getattr(tile.TileContext, "_drain_split_patched", False):
        return
    from concourse.tile import ScopedClock

    _orig_add = tile.TileContext._add_instruction

    def _patched_add(self, inst):
        si = getattr(inst, "sync_info", None)
        if si is not None and si.on_wait and len(si.on_wait) > 1:
            waits = list(si.on_wait)
            for i, w in enumerate(waits[:-1]):
                nop = mybir.InstNoOp(
                    name=f"{inst.name}-ws{i}",
                    sync_info=mybir.SyncInfo(on_wait=[w], on_update=[]),
                    bass_nofuse=True,
                    engine=inst.engine,
                )
                _orig_add(self, nop)
            si.on_wait = waits[-1:]
            inst.sync_info = si
        _orig_add(self, inst)

    tile.TileContext._add_instruction = _patched_add

    def _patched_dab(self, tick_clock, wait_clock):
        nc = self.nc
        drain_inst = nc.sync.drain()
        wait_clock.add_sem_waits(
            drain_inst.ins, ScopedClock({None: tick_clock.global_clock})
        )
        si = drain_inst.ins.sync_info
        if si is not None and si.on_wait and len(si.on_wait) > 1:
            waits = list(si.on_wait)
            si.on_wait = waits[:1]
            drain_inst.ins.sync_info = si
            for w in waits[1:]:
                d2 = nc.sync.drain()
                d2.ins.sync_info = mybir.SyncInfo(on_wait=[w], on_update=[])
        nc.all_engine_barrier()
        popped = nc._tile_sem_poison_stack.pop()
        assert popped is self._sem_poison
        nc.clear_and_free_semaphores(list(self.sems.allocated().values()))
        nc.all_engine_barrier()

    tile.TileContext._drain_and_barrier = _patched_dab
    tile.TileContext._drain_split_patched = True


def _emit(tc, out_d, x_d, wk_d, wq_d, wv_d):
    nc = tc.nc
    f32r = mybir.dt.float32r
    f32 = mybir.dt.float32
    bf16 = mybir.dt.bfloat16
    Exp = mybir.ActivationFunctionType.Exp

    CT = C // P  # 8 c-tiles
    TT = T // P  # 16 t-tiles
    BLKB = BLKB_W  # qkv block width
    BLK = 256  # attention block width
    H1 = H + 1
    scale = float(C) ** -0.5
    tdt = bf16  # vT/oT dtype: bf16 so their transposes use the bf16 identity

    with ExitStack() as ctx:
        const = ctx.enter_context(tc.tile_pool(name="const", bufs=1))
        persist = ctx.enter_context(tc.tile_pool(name="persist", bufs=1))
        xa_pool = ctx.enter_context(tc.tile_pool(name="xa", bufs=TT))
        pt_pool = ctx.enter_context(tc.tile_pool(name="ptp", bufs=PT_BUFS))
        oT_pool = ctx.enter_context(tc.tile_pool(name="otp", bufs=2))
        rec_pool = ctx.enter_context(tc.tile_pool(name="recp", bufs=2))
        # PSUM: 8 banks total so all phases can overlap.
        psA = ctx.enter_context(
            tc.tile_pool(name="psA", bufs=1 if PSA1_PSS3 else 2, space="PSUM")
        )
        psB = ctx.enter_context(tc.tile_pool(name="psB", bufs=1 if PSB1_PSS3 else 2, space="PSUM"))
        psS = ctx.enter_context(
            tc.tile_pool(
                name="psS",
                bufs=3 if (PSA1_PSS3 or PSB1_PSS3) else 2,
                space="PSUM",
            )
        )
        psO = ctx.enter_context(tc.tile_pool(name="psO", bufs=2, space="PSUM"))

        # identity: build in f32 (memset on f32r is invalid ISA in this
        # walrus); f32r and bf16 copies for dtype-matched transposes
        ident = const.tile([P, P], f32, name="ident")
        make_identity(nc, ident)
        identr = const.tile([P, P], f32r, name="identr")
        nc.vector.tensor_copy(out=identr, in_=ident)
        identb = const.tile([P, P], bf16, name="identb")
        nc.vector.tensor_copy(out=identb, in_=ident)
        idT = identb if IDENT_B else identr
        # 0/1 mask: mask[s, t] = 1 iff s <= t (keep causal entries)
        mask = const.tile([P, P], bf16, name="mask")
        nc.vector.memset(mask, 1.0)
        nc.gpsimd.affine_select(
            out=mask,
            in_=mask,
            compare_op=mybir.AluOpType.is_ge,
            fill=0.0,
            base=0,
            pattern=[[1, P]],
            channel_multiplier=-1,
        )

        # [Wk | Wq] packed: one M=128 matmul produces k on partitions 0-63
        # and q on 64-127
        wkq_sb = const.tile([P, CT, 2 * H], f32r, name="wkq_sb")
        wv_sb = const.tile([P, CT, H], f32r, name="wv_sb")

        xT = persist.tile([P, CT, T], f32r, name="xT")
        kT = persist.tile([H, T], f32r, name="kT")
        qT = persist.tile([H, T], f32r, name="qT")
        vT = persist.tile([H, T], tdt, name="vT")
        v1 = persist.tile([P, TT, H1], bf16, name="v1")
        out_sb = persist.tile([P, TT, H], f32, name="out_sb")

        nc.vector.memset(v1[:, :, H : H + 1], 1.0)

        # ---- DMA emission (SP stream order == DMA device service order).
        wk_r = wk_d.rearrange("(o p) h -> p o h", p=P)
        wq_r = wq_d.rearrange("(o p) h -> p o h", p=P)
        wv_r = wv_d.rearrange("(o p) h -> p o h", p=P)
        wslots = {
            0: {3: 0, 4: 1, 5: 2, 6: 3},
            1: {2: 0, 3: 1, 4: 2, 5: 3},
            2: {1: 0, 2: 1, 3: 2, 4: 3},
        }[W_SLOT]
        xa_tiles = []
        for tt in range(TT):
            tsl = slice(tt * P, (tt + 1) * P)
            xa = xa_pool.tile([P, C], f32r, name="xa")
            xa_tiles.append(xa)
            if tt < X_SPLIT:
                nc.sync.dma_start(xa[:, 0 : C // 2], x_d[tsl, 0 : C // 2])
                nc.sync.dma_start(xa[:, C // 2 : C], x_d[tsl, C // 2 : C])
            else:
                nc.sync.dma_start(xa, x_d[tsl, :])
            w = wslots.get(tt)
            if False:
                if w == 0:
                    nc.sync.dma_start(wkq_sb[:, :, 0:H], wk_r)
                elif w == 1:
                    nc.sync.dma_start(wkq_sb[:, :, H : 2 * H], wq_r)
                elif w == 2:
                    nc.sync.dma_start(wv_sb, wv_r)
            elif w is not None:
                if w == 0:
                    nc.sync.dma_start(wkq_sb[:, 0:4, 0:H], wk_r[:, 0:4, :])
                    nc.sync.dma_start(
                        wkq_sb[:, 0:4, H : 2 * H], wq_r[:, 0:4, :]
                    )
                elif w == 1:
                    nc.sync.dma_start(wkq_sb[:, 4:CT, 0:H], wk_r[:, 4:CT, :])
                    nc.sync.dma_start(
                        wkq_sb[:, 4:CT, H : 2 * H], wq_r[:, 4:CT, :]
                    )
                elif w == 2:
                    nc.sync.dma_start(wv_sb[:, 0:4, :], wv_r[:, 0:4, :])
                elif w == 3:
                    nc.sync.dma_start(wv_sb[:, 4:CT, :], wv_r[:, 4:CT, :])

        # ---- compute emission helpers
        copy_rr = [0]
        if XT_RATIO == 0:
            copy_engines = [nc.vector, nc.vector, nc.scalar, nc.vector]
        elif XT_RATIO == 1:
            copy_engines = [nc.vector, nc.scalar]
        else:
            copy_engines = [nc.vector]

        def phase_a(tt):
            """transpose x tile tt into xT."""
            tsl = slice(tt * P, (tt + 1) * P)
            xa = xa_tiles[tt]
            for cg in range(2):
                ps_t = psA.tile([P, 4, P], f32r, name="ps_t", tag="pst")
                for j in range(4):
                    ci = cg * 4 + j
                    nc.tensor.transpose(
                        ps_t[:, j, :], xa[:, ci * P : (ci + 1) * P], identr
                    )
                dst = xT[:, cg * 4 : cg * 4 + 4, tsl]
                eng = copy_engines[copy_rr[0] % len(copy_engines)]
                copy_rr[0] += 1
                if eng is nc.scalar:
                    nc.scalar.copy(out=dst, in_=ps_t)
                else:
                    eng.tensor_copy(out=dst, in_=ps_t)

        def phase_b_kq(c0, c1):
            tsl = slice(c0, c1)
            pkq = psB.tile([P, c1 - c0], f32, name="pkq", tag="bt")
            for ci in range(CT):
                nc.tensor.matmul(
                    pkq,
                    wkq_sb[:, ci, :],
                    xT[:, ci, tsl],
                    start=(ci == 0),
                    stop=(ci == CT - 1),
                )
            nc.scalar.copy(out=kT[:, tsl], in_=pkq[0:H, :])
            # partition-shift copy 64-127 -> 0-63 (legal on DVE)
            nc.vector.tensor_copy(out=qT[:, tsl], in_=pkq[H:P, :])

        def phase_b_v(c0, c1):
            tsl = slice(c0, c1)
            pv = psB.tile([H, c1 - c0], f32, name="pv", tag="bt")
            for ci in range(CT):
                nc.tensor.matmul(
                    pv,
                    wv_sb[:, ci, :],
                    xT[:, ci, tsl],
                    start=(ci == 0),
                    stop=(ci == CT - 1),
                )
            nc.scalar.copy(out=vT[:, tsl], in_=pv)
            for c4 in range((c1 - c0) // P):
                st = c0 // P + c4
                pvt = psB.tile([P, H], tdt, name="pvt", tag="bt")
                nc.tensor.transpose(
                    pvt, vT[:, st * P : (st + 1) * P], idT[:H, :H]
                )
                nc.vector.tensor_copy(out=v1[:, st, 0:H], in_=pvt)

        def c_block(cb, diag_first=False, mask_dve=False):
            pe_pool, pe_tag = (psA, "pst") if (
                PE_POOL_LAST and cb >= 6
            ) else (psO, "po")
            """attention block cb: S-pair(g) / PV-pair(g) / finish closures.
            The diagonal pair (g=cb) is emitted FIRST so its mask chain is
            off the block's critical tail; PSUM start/stop flags follow
            emission order (accumulation order is irrelevant to the sum)."""
            tsl = slice(cb * BLK, (cb + 1) * BLK)
            state = {}
            order = [cb] + list(range(cb)) if diag_first else list(range(cb + 1))
            first_pair = order[0]
            last_pair = order[-1]

            def start():
                state["po"] = psO.tile([H1, BLK], f32, name="po", tag="po")
                state["pt"] = {}

            def s_pair(g):
                ps_s = psS.tile([P, 2, BLK], f32, name="ps_s")
                for j in range(2):
                    st = 2 * g + j
                    nc.tensor.matmul(
                        ps_s[:, j, :],
                        kT[:, st * P : (st + 1) * P],
                        qT[:, tsl],
                        start=True,
                        stop=True,
                    )
                ptile = pt_pool.tile([P, 2, BLK], bf16, name="ptile")
                state["pt"][g] = ptile
                if g < cb:
                    nc.scalar.activation(ptile, ps_s, Exp, scale=scale)
                else:
                    nc.scalar.activation(
                        ptile[:, 0, :], ps_s[:, 0, :], Exp, scale=scale
                    )
                    nc.scalar.activation(
                        ptile[:, 1, P:BLK], ps_s[:, 1, P:BLK], Exp, scale=scale
                    )
                    nc.gpsimd.memset(ptile[:, 1, 0:P], 0.0)
                    meng = nc.vector if mask_dve else nc.gpsimd
                    meng.tensor_mul(ptile[:, 0, 0:P], ptile[:, 0, 0:P], mask)
                    meng.tensor_mul(
                        ptile[:, 1, P:BLK], ptile[:, 1, P:BLK], mask
                    )

            def pv_pair(g):
                po = state["po"]
                ptile = state["pt"].pop(g)
                for j in range(2):
                    st = 2 * g + j
                    nc.tensor.matmul(
                        po,
                        v1[:, st, 0:H1],
                        ptile[:, j, :],
                        start=(g == first_pair and j == 0),
                        stop=(g == last_pair and j == 1),
                    )

            def finish():
                po = state["po"]
                oT = oT_pool.tile([H1, BLK], tdt, name="oT")
                for c4 in range(BLK // P):
                    csl = slice(c4 * P, (c4 + 1) * P)
                    if c4 % 2 == 0:
                        nc.vector.tensor_copy(out=oT[:, csl], in_=po[:, csl])
                    else:
                        nc.scalar.copy(out=oT[:, csl], in_=po[:, csl])
                    st = cb * (BLK // P) + c4
                    pe = pe_pool.tile([P, H1], tdt, name="pe", tag=pe_tag)
                    nc.tensor.transpose(pe, oT[:, csl], idT[:H1, :H1])
                    rec = rec_pool.tile([P, 1], f32, name="rec")
                    nc.vector.reciprocal(rec, pe[:, H:H1])
                    nc.vector.tensor_scalar_mul(
                        out_sb[:, st, :], pe[:, 0:H], rec
                    )
                    nc.sync.dma_start(
                        out_d.rearrange("(o p) h -> p o h", p=P)[:, st, :],
                        out_sb[:, st, :],
                    )

            return start, s_pair, pv_pair, finish, order

        def phase_c_group(cbs, pv_offset, diag_first_last=False,
                          mask_dve=False):
            """emit several C blocks with all their S-pairs zipped ahead of
            the PV-pairs (offset in pairs) so the exp stream saturates ACT."""
            blocks = {}
            seq = []
            for cb in cbs:
                df = diag_first_last and cb == cbs[-1]
                start, s_pair, pv_pair, finish, order = c_block(
                    cb, df, mask_dve
                )
                start()
                blocks[cb] = (s_pair, pv_pair, finish)
                seq.extend((cb, g) for g in order)
            done = 0
            for i, (cb, g) in enumerate(seq):
                blocks[cb][0](g)
                if i >= pv_offset:
                    pcb, pg = seq[i - pv_offset]
                    blocks[pcb][1](pg)
                    done = i - pv_offset + 1
                    if done < len(seq) and seq[done][0] != pcb and (
                        done == 0 or seq[done - 1][0] == pcb
                    ):
                        blocks[pcb][2]()
            for i in range(done, len(seq)):
                pcb, pg = seq[i]
                blocks[pcb][1](pg)
                if i + 1 == len(seq) or seq[i + 1][0] != pcb:
                    blocks[pcb][2]()

        def phase_c(cb, diag_first=False, mask_dve=False):
            start, s_pair, pv_pair, finish, order = c_block(
                cb, diag_first, mask_dve
            )
            start()
            for i, g in enumerate(order):
                s_pair(g)
                if i >= PV_OFFSET:
                    pv_pair(order[i - PV_OFFSET])
            for i in range(max(0, len(order) - PV_OFFSET), len(order)):
                pv_pair(order[i])
            finish()

        def emit(tokens):
            for t in tokens.split():
                if t.startswith("A"):
                    phase_a(int(t[1:]))
                elif t.startswith("K"):
                    c0, c1 = t[1:].split("-")
                    phase_b_kq(int(c0), int(c1))
                elif t.startswith("V"):
                    c0, c1 = t[1:].split("-")
                    phase_b_v(int(c0), int(c1))
                elif t.startswith("C"):
                    cb = int(t[1:])
                    phase_c(cb, diag_first=(cb == 7 and DIAG_LAST7),
                            mask_dve=(cb >= 6))

        if LADDER == 9:
            phase_a(0)
            phase_a(1)
            phase_a(2)
            phase_a(3)
            phase_b_kq(0, 512)
            phase_b_v(0, 512)
            phase_c_group([0, 1], GRP_OFF)
            phase_a(4)
            phase_a(5)
            phase_a(6)
            phase_a(7)
            phase_b_kq(512, 1024)
            phase_b_v(512, 1024)
            phase_c_group([2, 3], GRP_OFF)
            phase_a(8)
            phase_a(9)
            phase_a(10)
            phase_a(11)
            phase_b_kq(1024, 1536)
            phase_b_v(1024, 1536)
            phase_c_group([4, 5], GRP_OFF)
            phase_a(12)
            phase_a(13)
            phase_b_kq(1536, 1792)
            phase_b_v(1536, 1792)
            phase_c(6, mask_dve=True)
            phase_a(14)
            phase_a(15)
            phase_b_kq(1792, 2048)
            phase_b_v(1792, 2048)
            phase_c(7, diag_first=DIAG_LAST7, mask_dve=True)
            return

        if LADDER == 8:
            phase_a(0)
            phase_a(1)
            phase_a(2)
            phase_a(3)
            phase_b_kq(0, 512)
            phase_a(4)
            phase_b_v(0, 512)
            phase_a(5)
            phase_c_group([0, 1], GRP_OFF)
            phase_a(6)
            phase_a(7)
            phase_b_kq(512, 1024)
            phase_a(8)
            phase_b_v(512, 1024)
            phase_a(9)
            phase_c_group([2, 3], GRP_OFF)
            phase_a(10)
            phase_a(11)
            phase_b_kq(1024, 1536)
            phase_a(12)
            phase_b_v(1024, 1536)
            phase_a(13)
            phase_c_group([4, 5], GRP_OFF)
            phase_a(14)
            phase_a(15)
            phase_b_kq(1536, 1792)
            phase_b_v(1536, 1792)
            phase_b_kq(1792, 2048)
            phase_b_v(1792, 2048)
            phase_c_group([6, 7], GRP_OFF, diag_first_last=DIAG_LAST7,
                          mask_dve=True)
            return

        LADDERS = {
            # best-known v2-style interleave with split last B block
            1: "A0 A1 A2 A3 K0-512 A4 V0-512 A5 C0 C1 A6 A7 "
               "K512-1024 A8 V512-1024 A9 C2 A10 C3 A11 "
               "K1024-1536 A12 V1024-1536 A13 C4 A14 C5 A15 "
               "K1536-1792 V1536-1792 C6 K1792-2048 V1792-2048 C7",
            # C-first everywhere
            2: "A0 A1 A2 A3 K0-512 V0-512 C0 C1 A4 A5 A6 A7 "
               "K512-1024 V512-1024 C2 C3 A8 A9 A10 A11 "
               "K1024-1536 V1024-1536 C4 C5 A12 A13 "
               "K1536-1792 V1536-1792 C6 A14 A15 K1792-2048 V1792-2048 C7",
            # hybrid: keep early interleave, pull late C blocks ahead of A14/A15
            3: "A0 A1 A2 A3 K0-512 A4 V0-512 A5 C0 C1 A6 A7 "
               "K512-1024 A8 V512-1024 A9 C2 A10 C3 A11 "
               "K1024-1536 A12 V1024-1536 A13 C4 C5 A14 A15 "
               "K1536-1792 V1536-1792 C6 K1792-2048 V1792-2048 C7",
            # hybrid + C4 before A13
            4: "A0 A1 A2 A3 K0-512 A4 V0-512 A5 C0 C1 A6 A7 "
               "K512-1024 A8 V512-1024 A9 C2 A10 C3 A11 "
               "K1024-1536 A12 V1024-1536 C4 A13 C5 A14 A15 "
               "K1536-1792 V1536-1792 C6 K1792-2048 V1792-2048 C7",
            # L1 + C2/C3 earlier relative to A9-A11
            5: "A0 A1 A2 A3 K0-512 A4 V0-512 A5 C0 C1 A6 A7 "
               "K512-1024 A8 V512-1024 C2 A9 C3 A10 A11 "
               "K1024-1536 A12 V1024-1536 A13 C4 A14 C5 A15 "
               "K1536-1792 V1536-1792 C6 K1792-2048 V1792-2048 C7",
            7: "A0 A1 A2 A3 K0-256 V0-256 C0 K256-512 V256-512 C1 A4 A5 "
               "K512-768 V512-768 C2 A6 A7 K768-1024 V768-1024 C3 A8 A9 "
               "K1024-1280 V1024-1280 C4 A10 A11 K1280-1536 V1280-1536 C5 "
               "A12 A13 K1536-1792 V1536-1792 C6 A14 A15 "
               "K1792-2048 V1792-2048 C7",
            # L5 + L4 combined
            6: "A0 A1 A2 A3 K0-512 A4 V0-512 A5 C0 C1 A6 A7 "
               "K512-1024 A8 V512-1024 C2 A9 C3 A10 A11 "
               "K1024-1536 A12 V1024-1536 C4 A13 C5 A14 A15 "
               "K1536-1792 V1536-1792 C6 K1792-2048 V1792-2048 C7",
        }
        emit(LADDERS[LADDER])


_NC_CACHE = {}


def build_nc():
    if "nc" in _NC_CACHE:
        return _NC_CACHE["nc"]
    _patch_drain_split()
    f32r = mybir.dt.float32r
    f32 = mybir.dt.float32
    nc = bass.Bass(
        "TRN2", target_bir_lowering=False, debug=False, num_devices=N_CORES
    )
    x_d = nc.dram_tensor("x", [T, C], f32r, kind="ExternalInput").ap()
    wk_d = nc.dram_tensor("Wk", [C, H], f32r, kind="ExternalInput").ap()
    wq_d = nc.dram_tensor("Wq", [C, H], f32r, kind="ExternalInput").ap()
    wv_d = nc.dram_tensor("Wv", [C, H], f32r, kind="ExternalInput").ap()
    out_d = nc.dram_tensor("out", [T, H], f32, kind="ExternalOutput").ap()
    with tile.TileContext(nc) as tc:
        _emit(tc, out_d, x_d, wk_d, wq_d, wv_d)
    _NC_CACHE["nc"] = nc
    return nc


def kernel(x, Wk, Wq, Wv, **run_kwargs):
    """Full-input entry point: shard over batch, run on cores 0-7, gather."""
    x = np.ascontiguousarray(np.asarray(x), dtype=np.float32)
    Wk = np.ascontiguousarray(np.asarray(Wk), dtype=np.float32)
    Wq = np.ascontiguousarray(np.asarray(Wq), dtype=np.float32)
    Wv = np.ascontiguousarray(np.asarray(Wv), dtype=np.float32)
    assert x.shape == (B, T, C), x.shape

    nc = build_nc()
    in_maps = [
        {"x": np.ascontiguousarray(x[b]), "Wk": Wk, "Wq": Wq, "Wv": Wv}
        for b in range(B)
    ]
    res = bass_utils.run_bass_kernel_spmd(
        nc, in_maps, core_ids=list(range(N_CORES)), **run_kwargs
    )
    out = np.stack([res.results[b]["out"] for b in range(B)], axis=0)
    if run_kwargs:
        kernel.last_results = res
    return out.astype(np.float32)


# revision 31
# speedup vs baseline: 1.0273x; 1.0223x over previous
"""nn_Head single-head causal attention on 8 TRN2 NeuronCores.

Full inputs: x [8, 2048, 1024] f32, Wk/Wq/Wv [1024, 64] f32.
Full output: [8, 2048, 64] f32 = softmax(causal(q k^T * C^-0.5)) @ v per batch.

Sharding: data-parallel over batch B=8 -> one batch element per core;
weights replicated. No collectives.

Per-core kernel (Bass/Tile, f32r matmuls + bf16 probability/value stage):
  A) x t-tiles DMA in (first two tiles split in halves to cut first-arrival
     latency); PE-transpose to xT [c-part, t] (TensorE contracts over the
     partition dim, fp32 cannot DMA-transpose).  All PE transposes use a
     bf16 identity as the moving operand: the moving dtype sets the
     cycles/row (bf16 1.0 vs f32r 1.5 / f32 2.0), while values stay exact.
  B) QKV per 512-col block: kT/qT/vT [h(64), t] via lhsT=W [c,64], rhs=xT;
     v transposed back to natural v1 [s-part, t-tile, H+1] bf16 with a ones
     column at H that makes the PV matmul also produce the softmax
     denominator.  W is DMAed in half-tensors interleaved between x tiles.
  C) attention per 256-col block: S^T tile = kT_slice^T@qT -> PSUM
     [s 128, t 256]; exp on ScalarE with scale=C^-0.5 folded in (scores are
     O(1): no max-subtraction needed, mathematically identical softmax);
     causality via memset of fully-masked column ranges + a 0/1
     upper-triangular mask mul on diagonal tiles; PV: po[h|denom, t] +=
     v1_slice^T @ P^T accumulated over s-tiles; po -> SBUF (f32r),
     PE-transpose to [t-part, H+1], multiply by per-partition reciprocal of
     the denominator column, streaming output DMAs.
  Emission interleaves B/C blocks between A tiles in data-arrival order so
  the PE always has ready work (each engine has a 4-deep scoreboard that
  lets ready instructions pass blocked ones).
"""

from contextlib import ExitStack

import numpy as np

import concourse.bass as bass
import concourse.mybir as mybir
import concourse.tile as tile
from concourse import bass_utils
from concourse.masks import make_identity

B, T, C, H = 8, 2048, 1024, 64
N_CORES = 8
P = 128

# ---- tuning knobs (swept via TimelineSim) ----
IDENT_B = True  # bf16 identity for all PE transposes
ACT_CLEAN = True  # keep ScalarE free for exp: xT/kT copies on Pool/DVE
PSA1_PSS3 = False  # psA bufs 1 / psS bufs 3 (vs 2/2)
W_EARLY = False  # W DMA halves at tt1/2/3 (vs tt3/4/5/6)
OUT_TILE = True  # per-tile output DMAs (vs per-block)
ZIP67 = False  # zip the last two attention blocks pair-by-pair
BLKB_W = 512  # qkv block width
SCHED = "v2"  # "v2" ladder or "weave"
PV_OFFSET = 2  # emit PV pairs this many pairs behind S pairs
DIAG_FIRST = False  # emit the diagonal pair first within each C block
PSB1_PSS3 = False  # psB bufs 1, psS bufs 3
LADDER = 8  # emission order variant
DIAG_LAST7 = True  # diagonal pair first in the final block
PT_BUFS = 20  # ptile pool depth
GRP_OFF = 5  # PV offset within a grouped C emission
XT_RATIO = 1  # xT copy engine mix: 0=3:1 DVE:ACT, 1=1:1, 2=all DVE
W_SLOT = 0  # which x-tile slots carry the W DMA halves
PE_POOL_LAST = False  # late C blocks take pe tiles from the idle psA pool
X_SPLIT = 16  # how many leading x tiles DMA in half-tiles
LATE_T = 10  # from this x-tile / col>=1024 on, copies avoid ScalarE
EPI_PAR = False  # parallel DVE/ACT epilogue chunk chains


def _patch_drain_split():
    """This walrus build accepts only one sem wait per instruction ("Too many
    sync wait commands" in setupSyncWait otherwise). Hoist extra waits onto
    same-engine NOPs ahead of the instruction (engine streams dispatch
    in-order, so the blocking semantics are identical), and split the
    TileContext tail drain the same way."""
    if getattr(tile.TileContext, "_drain_split_patched", False):
        return
    from concourse.tile import ScopedClock

    _orig_add = tile.TileContext._add_instruction

    def _patched_add(self, inst):
        si = getattr(inst, "sync_info", None)
        if si is not None and si.on_wait and len(si.on_wait) > 1:
            waits = list(si.on_wait)
            for i, w in enumerate(waits[:-1]):
                nop = mybir.InstNoOp(
                    name=f"{inst.name}-ws{i}",
                    sync_info=mybir.SyncInfo(on_wait=[w], on_update=[]),
                    bass_nofuse=True,
                    engine=inst.engine,
                )
                _orig_add(self, nop)
            si.on_wait = waits[-1:]
            inst.sync_info = si
        _orig_add(self, inst)

    tile.TileContext._add_instruction = _patched_add

    def _patched_dab(self, tick_clock, wait_clock):
        nc = self.nc
        drain_inst = nc.sync.drain()
        wait_clock.add_sem_waits(
            drain_inst.ins, ScopedClock({None: tick_clock.global_clock})
        )
        si = drain_inst.ins.sync_info
        if si is not None and si.on_wait and len(si.on_wait) > 1:
            waits = list(si.on_wait)
            si.on_wait = waits[:1]
            drain_inst.ins.sync_info = si
            for w in waits[1:]:
                d2 = nc.sync.drain()
                d2.ins.sync_info = mybir.SyncInfo(on_wait=[w], on_update=[])
        nc.all_engine_barrier()
        popped = nc._tile_sem_poison_stack.pop()
        assert popped is self._sem_poison
        nc.clear_and_free_semaphores(list(self.sems.allocated().values()))
        nc.all_engine_barrier()

    tile.TileContext._drain_and_barrier = _patched_dab
    tile.TileContext._drain_split_patched = True


def _emit(tc, out_d, x_d, wk_d, wq_d, wv_d):
    nc = tc.nc
    f32r = mybir.dt.float32r
    f32 = mybir.dt.float32
    bf16 = mybir.dt.bfloat16
    Exp = mybir.ActivationFunctionType.Exp

    CT = C // P  # 8 c-tiles
    TT = T // P  # 16 t-tiles
    BLKB = BLKB_W  # qkv block width
    BLK = 256  # attention block width
    H1 = H + 1
    scale = float(C) ** -0.5
    tdt = bf16  # vT/oT dtype: bf16 so their transposes use the bf16 identity

    with ExitStack() as ctx:
        const = ctx.enter_context(tc.tile_pool(name="const", bufs=1))
        persist = ctx.enter_context(tc.tile_pool(name="persist", bufs=1))
        xa_pool = ctx.enter_context(tc.tile_pool(name="xa", bufs=TT))
        pt_pool = ctx.enter_context(tc.tile_pool(name="ptp", bufs=PT_BUFS))
        oT_pool = ctx.enter_context(tc.tile_pool(name="otp", bufs=2))
        rec_pool = ctx.enter_context(tc.tile_pool(name="recp", bufs=2))
        # PSUM: 8 banks total so all phases can overlap.
        psA = ctx.enter_context(
            tc.tile_pool(name="psA", bufs=1 if PSA1_PSS3 else 2, space="PSUM")
        )
        psB = ctx.enter_context(tc.tile_pool(name="psB", bufs=1 if PSB1_PSS3 else 2, space="PSUM"))
        psS = ctx.enter_context(
            tc.tile_pool(
                name="psS",
                bufs=3 if (PSA1_PSS3 or PSB1_PSS3) else 2,
                space="PSUM",
            )
        )
        psO = ctx.enter_context(tc.tile_pool(name="psO", bufs=2, space="PSUM"))

        # identity: build in f32 (memset on f32r is invalid ISA in this
        # walrus); f32r and bf16 copies for dtype-matched transposes
        ident = const.tile([P, P], f32, name="ident")
        make_identity(nc, ident)
        identr = const.tile([P, P], f32r, name="identr")
        nc.vector.tensor_copy(out=identr, in_=ident)
        identb = const.tile([P, P], bf16, name="identb")
        nc.vector.tensor_copy(out=identb, in_=ident)
        idT = identb if IDENT_B else identr
        # 0/1 mask: mask[s, t] = 1 iff s <= t (keep causal entries)
        mask = const.tile([P, P], bf16, name="mask")
        nc.vector.memset(mask, 1.0)
        nc.gpsimd.affine_select(
            out=mask,
            in_=mask,
            compare_op=mybir.AluOpType.is_ge,
            fill=0.0,
            base=0,
            pattern=[[1, P]],
            channel_multiplier=-1,
        )

        # [Wk | Wq] packed: one M=128 matmul produces k on partitions 0-63
        # and q on 64-127
        wkq_sb = const.tile([P, CT, 2 * H], f32r, name="wkq_sb")
        wv_sb = const.tile([P, CT, H], f32r, name="wv_sb")

        xT = persist.tile([P, CT, T], f32r, name="xT")
        kT = persist.tile([H, T], f32r, name="kT")
        qT = persist.tile([H, T], f32r, name="qT")
        vT = persist.tile([H, T], tdt, name="vT")
        v1 = persist.tile([P, TT, H1], bf16, name="v1")
        out_sb = persist.tile([P, TT, H], f32, name="out_sb")

        nc.vector.memset(v1[:, :, H : H + 1], 1.0)

        # ---- DMA emission (SP stream order == DMA device service order).
        wk_r = wk_d.rearrange("(o p) h -> p o h", p=P)
        wq_r = wq_d.rearrange("(o p) h -> p o h", p=P)
        wv_r = wv_d.rearrange("(o p) h -> p o h", p=P)
        wslots = {
            0: {3: 0, 4: 1, 5: 2, 6: 3},
            1: {2: 0, 3: 1, 4: 2, 5: 3},
            2: {1: 0, 2: 1, 3: 2, 4: 3},
        }[W_SLOT]
        xa_tiles = []
        for tt in range(TT):
            tsl = slice(tt * P, (tt + 1) * P)
            xa = xa_pool.tile([P, C], f32r, name="xa")
            xa_tiles.append(xa)
            if tt < X_SPLIT:
                nc.sync.dma_start(xa[:, 0 : C // 2], x_d[tsl, 0 : C // 2])
                nc.sync.dma_start(xa[:, C // 2 : C], x_d[tsl, C // 2 : C])
            else:
                nc.sync.dma_start(xa, x_d[tsl, :])
            w = wslots.get(tt)
            if False:
                if w == 0:
                    nc.sync.dma_start(wkq_sb[:, :, 0:H], wk_r)
                elif w == 1:
                    nc.sync.dma_start(wkq_sb[:, :, H : 2 * H], wq_r)
                elif w == 2:
                    nc.sync.dma_start(wv_sb, wv_r)
            elif w is not None:
                if w == 0:
                    nc.sync.dma_start(wkq_sb[:, 0:4, 0:H], wk_r[:, 0:4, :])
                    nc.sync.dma_start(
                        wkq_sb[:, 0:4, H : 2 * H], wq_r[:, 0:4, :]
                    )
                elif w == 1:
                    nc.sync.dma_start(wkq_sb[:, 4:CT, 0:H], wk_r[:, 4:CT, :])
                    nc.sync.dma_start(
                        wkq_sb[:, 4:CT, H : 2 * H], wq_r[:, 4:CT, :]
                    )
                elif w == 2:
                    nc.sync.dma_start(wv_sb[:, 0:4, :], wv_r[:, 0:4, :])
                elif w == 3:
                    nc.sync.dma_start(wv_sb[:, 4:CT, :], wv_r[:, 4:CT, :])

        # ---- compute emission helpers
        copy_rr = [0]
        if XT_RATIO == 0:
            copy_engines = [nc.vector, nc.vector, nc.scalar, nc.vector]
        elif XT_RATIO == 1:
            copy_engines = [nc.vector, nc.scalar]
        else:
            copy_engines = [nc.vector]

        def phase_a(tt):
            """transpose x tile tt into xT."""
            tsl = slice(tt * P, (tt + 1) * P)
            xa = xa_tiles[tt]
            for cg in range(2):
                ps_t = psA.tile([P, 4, P], f32r, name="ps_t", tag="pst")
                for j in range(4):
                    ci = cg * 4 + j
                    nc.tensor.transpose(
                        ps_t[:, j, :], xa[:, ci * P : (ci + 1) * P], identr
                    )
                dst = xT[:, cg * 4 : cg * 4 + 4, tsl]
                if tt >= LATE_T:
                    eng = nc.vector
                else:
                    eng = copy_engines[copy_rr[0] % len(copy_engines)]
                copy_rr[0] += 1
                if eng is nc.scalar:
                    nc.scalar.copy(out=dst, in_=ps_t)
                else:
                    eng.tensor_copy(out=dst, in_=ps_t)

        def phase_b_kq(c0, c1):
            tsl = slice(c0, c1)
            pkq = psB.tile([P, c1 - c0], f32, name="pkq", tag="bt")
            for ci in range(CT):
                nc.tensor.matmul(
                    pkq,
                    wkq_sb[:, ci, :],
                    xT[:, ci, tsl],
                    start=(ci == 0),
                    stop=(ci == CT - 1),
                )
            if c0 >= LATE_T * P:
                nc.vector.tensor_copy(out=kT[:, tsl], in_=pkq[0:H, :])
            else:
                nc.scalar.copy(out=kT[:, tsl], in_=pkq[0:H, :])
            # partition-shift copy 64-127 -> 0-63 (legal on DVE)
            nc.vector.tensor_copy(out=qT[:, tsl], in_=pkq[H:P, :])

        def phase_b_v(c0, c1):
            tsl = slice(c0, c1)
            pv = psB.tile([H, c1 - c0], f32, name="pv", tag="bt")
            for ci in range(CT):
                nc.tensor.matmul(
                    pv,
                    wv_sb[:, ci, :],
                    xT[:, ci, tsl],
                    start=(ci == 0),
                    stop=(ci == CT - 1),
                )
            if c0 >= LATE_T * P:
                nc.vector.tensor_copy(out=vT[:, tsl], in_=pv)
            else:
                nc.scalar.copy(out=vT[:, tsl], in_=pv)
            for c4 in range((c1 - c0) // P):
                st = c0 // P + c4
                pvt = psB.tile([P, H], tdt, name="pvt", tag="bt")
                nc.tensor.transpose(
                    pvt, vT[:, st * P : (st + 1) * P], idT[:H, :H]
                )
                nc.vector.tensor_copy(out=v1[:, st, 0:H], in_=pvt)

        def c_block(cb, diag_first=False, mask_dve=False):
            pe_pool, pe_tag = (psA, "pst") if (
                PE_POOL_LAST and cb >= 6
            ) else (psO, "po")
            """attention block cb: S-pair(g) / PV-pair(g) / finish closures.
            The diagonal pair (g=cb) is emitted FIRST so its mask chain is
            off the block's critical tail; PSUM start/stop flags follow
            emission order (accumulation order is irrelevant to the sum)."""
            tsl = slice(cb * BLK, (cb + 1) * BLK)
            state = {}
            order = [cb] + list(range(cb)) if diag_first else list(range(cb + 1))
            first_pair = order[0]
            last_pair = order[-1]

            def start():
                state["po"] = psO.tile([H1, BLK], f32, name="po", tag="po")
                state["pt"] = {}

            def s_pair(g):
                ps_s = psS.tile([P, 2, BLK], f32, name="ps_s")
                for j in range(2):
                    st = 2 * g + j
                    nc.tensor.matmul(
                        ps_s[:, j, :],
                        kT[:, st * P : (st + 1) * P],
                        qT[:, tsl],
                        start=True,
                        stop=True,
                    )
                ptile = pt_pool.tile([P, 2, BLK], bf16, name="ptile")
                state["pt"][g] = ptile
                if g < cb:
                    nc.scalar.activation(ptile, ps_s, Exp, scale=scale)
                else:
                    nc.scalar.activation(
                        ptile[:, 0, :], ps_s[:, 0, :], Exp, scale=scale
                    )
                    nc.scalar.activation(
                        ptile[:, 1, P:BLK], ps_s[:, 1, P:BLK], Exp, scale=scale
                    )
                    nc.gpsimd.memset(ptile[:, 1, 0:P], 0.0)
                    meng = nc.vector if mask_dve else nc.gpsimd
                    meng.tensor_mul(ptile[:, 0, 0:P], ptile[:, 0, 0:P], mask)
                    meng.tensor_mul(
                        ptile[:, 1, P:BLK], ptile[:, 1, P:BLK], mask
                    )

            def pv_pair(g):
                po = state["po"]
                ptile = state["pt"].pop(g)
                for j in range(2):
                    st = 2 * g + j
                    nc.tensor.matmul(
                        po,
                        v1[:, st, 0:H1],
                        ptile[:, j, :],
                        start=(g == first_pair and j == 0),
                        stop=(g == last_pair and j == 1),
                    )

            def finish():
                po = state["po"]
                oT = oT_pool.tile([H1, BLK], tdt, name="oT")
                for c4 in range(BLK // P):
                    csl = slice(c4 * P, (c4 + 1) * P)
                    odd = c4 % 2 == 1
                    if odd and EPI_PAR:
                        nc.scalar.copy(out=oT[:, csl], in_=po[:, csl])
                    else:
                        nc.vector.tensor_copy(out=oT[:, csl], in_=po[:, csl])
                    st = cb * (BLK // P) + c4
                    pe = pe_pool.tile([P, H1], tdt, name="pe", tag=pe_tag)
                    nc.tensor.transpose(pe, oT[:, csl], idT[:H1, :H1])
                    rec = rec_pool.tile([P, 1], f32, name="rec")
                    nc.vector.reciprocal(rec, pe[:, H:H1])
                    if odd and EPI_PAR:
                        nc.scalar.mul(out_sb[:, st, :], pe[:, 0:H], rec)
                    else:
                        nc.vector.tensor_scalar_mul(
                            out_sb[:, st, :], pe[:, 0:H], rec
                        )
                    nc.sync.dma_start(
                        out_d.rearrange("(o p) h -> p o h", p=P)[:, st, :],
                        out_sb[:, st, :],
                    )

            return start, s_pair, pv_pair, finish, order

        def phase_c_group(cbs, pv_offset, diag_first_last=False,
                          mask_dve=False, inject=None):
            """emit several C blocks with all their S-pairs zipped ahead of
            the PV-pairs (offset in pairs) so the exp stream saturates ACT."""
            blocks = {}
            seq = []
            for cb in cbs:
                df = diag_first_last and cb == cbs[-1]
                start, s_pair, pv_pair, finish, order = c_block(
                    cb, df, mask_dve
                )
                start()
                blocks[cb] = (s_pair, pv_pair, finish)
                seq.extend((cb, g) for g in order)
            done = 0
            inject = inject or {}
            for i, (cb, g) in enumerate(seq):
                blocks[cb][0](g)
                if i in inject:
                    inject[i]()
                if i >= pv_offset:
                    pcb, pg = seq[i - pv_offset]
                    blocks[pcb][1](pg)
                    done = i - pv_offset + 1
                    if done < len(seq) and seq[done][0] != pcb and (
                        done == 0 or seq[done - 1][0] == pcb
                    ):
                        blocks[pcb][2]()
            for i in range(done, len(seq)):
                pcb, pg = seq[i]
                blocks[pcb][1](pg)
                if i + 1 == len(seq) or seq[i + 1][0] != pcb:
                    blocks[pcb][2]()

        def phase_c(cb, diag_first=False, mask_dve=False):
            start, s_pair, pv_pair, finish, order = c_block(
                cb, diag_first, mask_dve
            )
            start()
            for i, g in enumerate(order):
                s_pair(g)
                if i >= PV_OFFSET:
                    pv_pair(order[i - PV_OFFSET])
            for i in range(max(0, len(order) - PV_OFFSET), len(order)):
                pv_pair(order[i])
            finish()

        def emit(tokens):
            for t in tokens.split():
                if t.startswith("A"):
                    phase_a(int(t[1:]))
                elif t.startswith("K"):
                    c0, c1 = t[1:].split("-")
                    phase_b_kq(int(c0), int(c1))
                elif t.startswith("V"):
                    c0, c1 = t[1:].split("-")
                    phase_b_v(int(c0), int(c1))
                elif t.startswith("C"):
                    cb = int(t[1:])
                    phase_c(cb, diag_first=(cb == 7 and DIAG_LAST7),
                            mask_dve=(cb >= 6))

        if LADDER == 10:
            phase_a(0)
            phase_a(1)
            phase_a(2)
            phase_a(3)
            phase_b_kq(0, 512)
            phase_c_group([0, 1], GRP_OFF,
                          inject={0: lambda: phase_b_v(0, 512)})
            phase_a(4)
            phase_a(5)
            phase_a(6)
            phase_a(7)
            phase_b_kq(512, 1024)
            phase_c_group([2, 3], GRP_OFF,
                          inject={1: lambda: phase_b_v(512, 1024)})
            phase_a(8)
            phase_a(9)
            phase_a(10)
            phase_a(11)
            phase_b_kq(1024, 1536)
            phase_c_group([4, 5], GRP_OFF,
                          inject={1: lambda: phase_b_v(1024, 1536)})
            phase_a(12)
            phase_a(13)
            phase_b_kq(1536, 1792)
            phase_c_group([6], GRP_OFF, mask_dve=True,
                          inject={1: lambda: phase_b_v(1536, 1792)})
            phase_a(14)
            phase_a(15)
            phase_b_kq(1792, 2048)
            phase_c_group([7], GRP_OFF, diag_first_last=DIAG_LAST7,
                          mask_dve=True,
                          inject={1: lambda: phase_b_v(1792, 2048)})
            return

        if LADDER == 9:
            phase_a(0)
            phase_a(1)
            phase_a(2)
            phase_a(3)
            phase_b_kq(0, 512)
            phase_b_v(0, 512)
            phase_c_group([0, 1], GRP_OFF)
            phase_a(4)
            phase_a(5)
            phase_a(6)
            phase_a(7)
            phase_b_kq(512, 1024)
            phase_b_v(512, 1024)
            phase_c_group([2, 3], GRP_OFF)
            phase_a(8)
            phase_a(9)
            phase_a(10)
            phase_a(11)
            phase_b_kq(1024, 1536)
            phase_b_v(1024, 1536)
            phase_c_group([4, 5], GRP_OFF)
            phase_a(12)
            phase_a(13)
            phase_b_kq(1536, 1792)
            phase_b_v(1536, 1792)
            phase_c(6, mask_dve=True)
            phase_a(14)
            phase_a(15)
            phase_b_kq(1792, 2048)
            phase_b_v(1792, 2048)
            phase_c(7, diag_first=DIAG_LAST7, mask_dve=True)
            return

        if LADDER == 8:
            phase_a(0)
            phase_a(1)
            phase_a(2)
            phase_a(3)
            phase_b_kq(0, 512)
            phase_a(4)
            phase_b_v(0, 512)
            phase_a(5)
            phase_c_group([0, 1], GRP_OFF)
            phase_a(6)
            phase_a(7)
            phase_b_kq(512, 1024)
            phase_a(8)
            phase_b_v(512, 1024)
            phase_a(9)
            phase_c_group([2, 3], GRP_OFF)
            phase_a(10)
            phase_a(11)
            phase_b_kq(1024, 1536)
            phase_a(12)
            phase_b_v(1024, 1536)
            phase_a(13)
            phase_c_group([4, 5], GRP_OFF)
            phase_a(14)
            phase_a(15)
            phase_b_kq(1536, 1792)
            phase_b_v(1536, 1792)
            phase_b_kq(1792, 2048)
            phase_b_v(1792, 2048)
            phase_c_group([6, 7], GRP_OFF, diag_first_last=DIAG_LAST7,
                          mask_dve=True)
            return

        LADDERS = {
            # best-known v2-style interleave with split last B block
            1: "A0 A1 A2 A3 K0-512 A4 V0-512 A5 C0 C1 A6 A7 "
               "K512-1024 A8 V512-1024 A9 C2 A10 C3 A11 "
               "K1024-1536 A12 V1024-1536 A13 C4 A14 C5 A15 "
               "K1536-1792 V1536-1792 C6 K1792-2048 V1792-2048 C7",
            # C-first everywhere
            2: "A0 A1 A2 A3 K0-512 V0-512 C0 C1 A4 A5 A6 A7 "
               "K512-1024 V512-1024 C2 C3 A8 A9 A10 A11 "
               "K1024-1536 V1024-1536 C4 C5 A12 A13 "
               "K1536-1792 V1536-1792 C6 A14 A15 K1792-2048 V1792-2048 C7",
            # hybrid: keep early interleave, pull late C blocks ahead of A14/A15
            3: "A0 A1 A2 A3 K0-512 A4 V0-512 A5 C0 C1 A6 A7 "
               "K512-1024 A8 V512-1024 A9 C2 A10 C3 A11 "
               "K1024-1536 A12 V1024-1536 A13 C4 C5 A14 A15 "
               "K1536-1792 V1536-1792 C6 K1792-2048 V1792-2048 C7",
            # hybrid + C4 before A13
            4: "A0 A1 A2 A3 K0-512 A4 V0-512 A5 C0 C1 A6 A7 "
               "K512-1024 A8 V512-1024 A9 C2 A10 C3 A11 "
               "K1024-1536 A12 V1024-1536 C4 A13 C5 A14 A15 "
               "K1536-1792 V1536-1792 C6 K1792-2048 V1792-2048 C7",
            # L1 + C2/C3 earlier relative to A9-A11
            5: "A0 A1 A2 A3 K0-512 A4 V0-512 A5 C0 C1 A6 A7 "
               "K512-1024 A8 V512-1024 C2 A9 C3 A10 A11 "
               "K1024-1536 A12 V1024-1536 A13 C4 A14 C5 A15 "
               "K1536-1792 V1536-1792 C6 K1792-2048 V1792-2048 C7",
            7: "A0 A1 A2 A3 K0-256 V0-256 C0 K256-512 V256-512 C1 A4 A5 "
               "K512-768 V512-768 C2 A6 A7 K768-1024 V768-1024 C3 A8 A9 "
               "K1024-1280 V1024-1280 C4 A10 A11 K1280-1536 V1280-1536 C5 "
               "A12 A13 K1536-1792 V1536-1792 C6 A14 A15 "
               "K1792-2048 V1792-2048 C7",
            # L5 + L4 combined
            6: "A0 A1 A2 A3 K0-512 A4 V0-512 A5 C0 C1 A6 A7 "
               "K512-1024 A8 V512-1024 C2 A9 C3 A10 A11 "
               "K1024-1536 A12 V1024-1536 C4 A13 C5 A14 A15 "
               "K1536-1792 V1536-1792 C6 K1792-2048 V1792-2048 C7",
        }
        emit(LADDERS[LADDER])


_NC_CACHE = {}


def build_nc():
    if "nc" in _NC_CACHE:
        return _NC_CACHE["nc"]
    _patch_drain_split()
    f32r = mybir.dt.float32r
    f32 = mybir.dt.float32
    nc = bass.Bass(
        "TRN2", target_bir_lowering=False, debug=False, num_devices=N_CORES
    )
    x_d = nc.dram_tensor("x", [T, C], f32r, kind="ExternalInput").ap()
    wk_d = nc.dram_tensor("Wk", [C, H], f32r, kind="ExternalInput").ap()
    wq_d = nc.dram_tensor("Wq", [C, H], f32r, kind="ExternalInput").ap()
    wv_d = nc.dram_tensor("Wv", [C, H], f32r, kind="ExternalInput").ap()
    out_d = nc.dram_tensor("out", [T, H], f32, kind="ExternalOutput").ap()
    with tile.TileContext(nc) as tc:
        _emit(tc, out_d, x_d, wk_d, wq_d, wv_d)
    _NC_CACHE["nc"] = nc
    return nc


def kernel(x, Wk, Wq, Wv, **run_kwargs):
    """Full-input entry point: shard over batch, run on cores 0-7, gather."""
    x = np.ascontiguousarray(np.asarray(x), dtype=np.float32)
    Wk = np.ascontiguousarray(np.asarray(Wk), dtype=np.float32)
    Wq = np.ascontiguousarray(np.asarray(Wq), dtype=np.float32)
    Wv = np.ascontiguousarray(np.asarray(Wv), dtype=np.float32)
    assert x.shape == (B, T, C), x.shape

    nc = build_nc()
    in_maps = [
        {"x": np.ascontiguousarray(x[b]), "Wk": Wk, "Wq": Wq, "Wv": Wv}
        for b in range(B)
    ]
    res = bass_utils.run_bass_kernel_spmd(
        nc, in_maps, core_ids=list(range(N_CORES)), **run_kwargs
    )
    out = np.stack([res.results[b]["out"] for b in range(B)], axis=0)
    if run_kwargs:
        kernel.last_results = res
    return out.astype(np.float32)


# revision 32
# speedup vs baseline: 1.0282x; 1.0009x over previous
"""nn_Head single-head causal attention on 8 TRN2 NeuronCores.

Full inputs: x [8, 2048, 1024] f32, Wk/Wq/Wv [1024, 64] f32.
Full output: [8, 2048, 64] f32 = softmax(causal(q k^T * C^-0.5)) @ v per batch.

Sharding: data-parallel over batch B=8 -> one batch element per core;
weights replicated. No collectives.

Per-core kernel (Bass/Tile, f32r matmuls + bf16 probability/value stage):
  A) x t-tiles DMA in (first two tiles split in halves to cut first-arrival
     latency); PE-transpose to xT [c-part, t] (TensorE contracts over the
     partition dim, fp32 cannot DMA-transpose).  All PE transposes use a
     bf16 identity as the moving operand: the moving dtype sets the
     cycles/row (bf16 1.0 vs f32r 1.5 / f32 2.0), while values stay exact.
  B) QKV per 512-col block: kT/qT/vT [h(64), t] via lhsT=W [c,64], rhs=xT;
     v transposed back to natural v1 [s-part, t-tile, H+1] bf16 with a ones
     column at H that makes the PV matmul also produce the softmax
     denominator.  W is DMAed in half-tensors interleaved between x tiles.
  C) attention per 256-col block: S^T tile = kT_slice^T@qT -> PSUM
     [s 128, t 256]; exp on ScalarE with scale=C^-0.5 folded in (scores are
     O(1): no max-subtraction needed, mathematically identical softmax);
     causality via memset of fully-masked column ranges + a 0/1
     upper-triangular mask mul on diagonal tiles; PV: po[h|denom, t] +=
     v1_slice^T @ P^T accumulated over s-tiles; po -> SBUF (f32r),
     PE-transpose to [t-part, H+1], multiply by per-partition reciprocal of
     the denominator column, streaming output DMAs.
  Emission interleaves B/C blocks between A tiles in data-arrival order so
  the PE always has ready work (each engine has a 4-deep scoreboard that
  lets ready instructions pass blocked ones).
"""

from contextlib import ExitStack

import numpy as np

import concourse.bass as bass
import concourse.mybir as mybir
import concourse.tile as tile
from concourse import bass_utils
from concourse.masks import make_identity

B, T, C, H = 8, 2048, 1024, 64
N_CORES = 8
P = 128

# ---- tuning knobs (swept via TimelineSim) ----
IDENT_B = True  # bf16 identity for all PE transposes
ACT_CLEAN = True  # keep ScalarE free for exp: xT/kT copies on Pool/DVE
PSA1_PSS3 = False  # psA bufs 1 / psS bufs 3 (vs 2/2)
W_EARLY = False  # W DMA halves at tt1/2/3 (vs tt3/4/5/6)
OUT_TILE = True  # per-tile output DMAs (vs per-block)
ZIP67 = False  # zip the last two attention blocks pair-by-pair
BLKB_W = 512  # qkv block width
SCHED = "v2"  # "v2" ladder or "weave"
PV_OFFSET = 2  # emit PV pairs this many pairs behind S pairs
DIAG_FIRST = False  # emit the diagonal pair first within each C block
PSB1_PSS3 = False  # psB bufs 1, psS bufs 3
LADDER = 8  # emission order variant
DIAG_LAST7 = True  # diagonal pair first in the final block
PT_BUFS = 20  # ptile pool depth
GRP_OFF = 5  # PV offset within a grouped C emission
XT_RATIO = 1  # xT copy engine mix: 0=3:1 DVE:ACT, 1=1:1, 2=all DVE
W_SLOT = 0  # which x-tile slots carry the W DMA halves
PE_POOL_LAST = False  # late C blocks take pe tiles from the idle psA pool
X_SPLIT = 16  # how many leading x tiles DMA in half-tiles
LATE_T = 9  # from this x-tile / col on, copies avoid ScalarE
EPI_PAR = False  # parallel DVE/ACT epilogue chunk chains


def _patch_drain_split():
    """This walrus build accepts only one sem wait per instruction ("Too many
    sync wait commands" in setupSyncWait otherwise). Hoist extra waits onto
    same-engine NOPs ahead of the instruction (engine streams dispatch
    in-order, so the blocking semantics are identical), and split the
    TileContext tail drain the same way."""
    if getattr(tile.TileContext, "_drain_split_patched", False):
        return
    from concourse.tile import ScopedClock

    _orig_add = tile.TileContext._add_instruction

    def _patched_add(self, inst):
        si = getattr(inst, "sync_info", None)
        if si is not None and si.on_wait and len(si.on_wait) > 1:
            waits = list(si.on_wait)
            for i, w in enumerate(waits[:-1]):
                nop = mybir.InstNoOp(
                    name=f"{inst.name}-ws{i}",
                    sync_info=mybir.SyncInfo(on_wait=[w], on_update=[]),
                    bass_nofuse=True,
                    engine=inst.engine,
                )
                _orig_add(self, nop)
            si.on_wait = waits[-1:]
            inst.sync_info = si
        _orig_add(self, inst)

    tile.TileContext._add_instruction = _patched_add

    def _patched_dab(self, tick_clock, wait_clock):
        nc = self.nc
        drain_inst = nc.sync.drain()
        wait_clock.add_sem_waits(
            drain_inst.ins, ScopedClock({None: tick_clock.global_clock})
        )
        si = drain_inst.ins.sync_info
        if si is not None and si.on_wait and len(si.on_wait) > 1:
            waits = list(si.on_wait)
            si.on_wait = waits[:1]
            drain_inst.ins.sync_info = si
            for w in waits[1:]:
                d2 = nc.sync.drain()
                d2.ins.sync_info = mybir.SyncInfo(on_wait=[w], on_update=[])
        nc.all_engine_barrier()
        popped = nc._tile_sem_poison_stack.pop()
        assert popped is self._sem_poison
        nc.clear_and_free_semaphores(list(self.sems.allocated().values()))
        nc.all_engine_barrier()

    tile.TileContext._drain_and_barrier = _patched_dab
    tile.TileContext._drain_split_patched = True


def _emit(tc, out_d, x_d, wk_d, wq_d, wv_d):
    nc = tc.nc
    f32r = mybir.dt.float32r
    f32 = mybir.dt.float32
    bf16 = mybir.dt.bfloat16
    Exp = mybir.ActivationFunctionType.Exp

    CT = C // P  # 8 c-tiles
    TT = T // P  # 16 t-tiles
    BLKB = BLKB_W  # qkv block width
    BLK = 256  # attention block width
    H1 = H + 1
    scale = float(C) ** -0.5
    tdt = bf16  # vT/oT dtype: bf16 so their transposes use the bf16 identity

    with ExitStack() as ctx:
        const = ctx.enter_context(tc.tile_pool(name="const", bufs=1))
        persist = ctx.enter_context(tc.tile_pool(name="persist", bufs=1))
        xa_pool = ctx.enter_context(tc.tile_pool(name="xa", bufs=TT))
        pt_pool = ctx.enter_context(tc.tile_pool(name="ptp", bufs=PT_BUFS))
        oT_pool = ctx.enter_context(tc.tile_pool(name="otp", bufs=2))
        rec_pool = ctx.enter_context(tc.tile_pool(name="recp", bufs=2))
        # PSUM: 8 banks total so all phases can overlap.
        psA = ctx.enter_context(
            tc.tile_pool(name="psA", bufs=1 if PSA1_PSS3 else 2, space="PSUM")
        )
        psB = ctx.enter_context(tc.tile_pool(name="psB", bufs=1 if PSB1_PSS3 else 2, space="PSUM"))
        psS = ctx.enter_context(
            tc.tile_pool(
                name="psS",
                bufs=3 if (PSA1_PSS3 or PSB1_PSS3) else 2,
                space="PSUM",
            )
        )
        psO = ctx.enter_context(tc.tile_pool(name="psO", bufs=2, space="PSUM"))

        # identity: build in f32 (memset on f32r is invalid ISA in this
        # walrus); f32r and bf16 copies for dtype-matched transposes
        ident = const.tile([P, P], f32, name="ident")
        make_identity(nc, ident)
        identr = const.tile([P, P], f32r, name="identr")
        nc.vector.tensor_copy(out=identr, in_=ident)
        identb = const.tile([P, P], bf16, name="identb")
        nc.vector.tensor_copy(out=identb, in_=ident)
        idT = identb if IDENT_B else identr
        # 0/1 mask: mask[s, t] = 1 iff s <= t (keep causal entries)
        mask = const.tile([P, P], bf16, name="mask")
        nc.vector.memset(mask, 1.0)
        nc.gpsimd.affine_select(
            out=mask,
            in_=mask,
            compare_op=mybir.AluOpType.is_ge,
            fill=0.0,
            base=0,
            pattern=[[1, P]],
            channel_multiplier=-1,
        )

        # [Wk | Wq] packed: one M=128 matmul produces k on partitions 0-63
        # and q on 64-127
        wkq_sb = const.tile([P, CT, 2 * H], f32r, name="wkq_sb")
        wv_sb = const.tile([P, CT, H], f32r, name="wv_sb")

        xT = persist.tile([P, CT, T], f32r, name="xT")
        kT = persist.tile([H, T], f32r, name="kT")
        qT = persist.tile([H, T], f32r, name="qT")
        vT = persist.tile([H, T], tdt, name="vT")
        v1 = persist.tile([P, TT, H1], bf16, name="v1")
        out_sb = persist.tile([P, TT, H], f32, name="out_sb")

        nc.vector.memset(v1[:, :, H : H + 1], 1.0)

        # ---- DMA emission (SP stream order == DMA device service order).
        wk_r = wk_d.rearrange("(o p) h -> p o h", p=P)
        wq_r = wq_d.rearrange("(o p) h -> p o h", p=P)
        wv_r = wv_d.rearrange("(o p) h -> p o h", p=P)
        wslots = {
            0: {3: 0, 4: 1, 5: 2, 6: 3},
            1: {2: 0, 3: 1, 4: 2, 5: 3},
            2: {1: 0, 2: 1, 3: 2, 4: 3},
        }[W_SLOT]
        xa_tiles = []
        for tt in range(TT):
            tsl = slice(tt * P, (tt + 1) * P)
            xa = xa_pool.tile([P, C], f32r, name="xa")
            xa_tiles.append(xa)
            if tt < X_SPLIT:
                nc.sync.dma_start(xa[:, 0 : C // 2], x_d[tsl, 0 : C // 2])
                nc.sync.dma_start(xa[:, C // 2 : C], x_d[tsl, C // 2 : C])
            else:
                nc.sync.dma_start(xa, x_d[tsl, :])
            w = wslots.get(tt)
            if False:
                if w == 0:
                    nc.sync.dma_start(wkq_sb[:, :, 0:H], wk_r)
                elif w == 1:
                    nc.sync.dma_start(wkq_sb[:, :, H : 2 * H], wq_r)
                elif w == 2:
                    nc.sync.dma_start(wv_sb, wv_r)
            elif w is not None:
                if w == 0:
                    nc.sync.dma_start(wkq_sb[:, 0:4, 0:H], wk_r[:, 0:4, :])
                    nc.sync.dma_start(
                        wkq_sb[:, 0:4, H : 2 * H], wq_r[:, 0:4, :]
                    )
                elif w == 1:
                    nc.sync.dma_start(wkq_sb[:, 4:CT, 0:H], wk_r[:, 4:CT, :])
                    nc.sync.dma_start(
                        wkq_sb[:, 4:CT, H : 2 * H], wq_r[:, 4:CT, :]
                    )
                elif w == 2:
                    nc.sync.dma_start(wv_sb[:, 0:4, :], wv_r[:, 0:4, :])
                elif w == 3:
                    nc.sync.dma_start(wv_sb[:, 4:CT, :], wv_r[:, 4:CT, :])

        # ---- compute emission helpers
        copy_rr = [0]
        if XT_RATIO == 0:
            copy_engines = [nc.vector, nc.vector, nc.scalar, nc.vector]
        elif XT_RATIO == 1:
            copy_engines = [nc.vector, nc.scalar]
        else:
            copy_engines = [nc.vector]

        def phase_a(tt):
            """transpose x tile tt into xT."""
            tsl = slice(tt * P, (tt + 1) * P)
            xa = xa_tiles[tt]
            for cg in range(2):
                ps_t = psA.tile([P, 4, P], f32r, name="ps_t", tag="pst")
                for j in range(4):
                    ci = cg * 4 + j
                    nc.tensor.transpose(
                        ps_t[:, j, :], xa[:, ci * P : (ci + 1) * P], identr
                    )
                dst = xT[:, cg * 4 : cg * 4 + 4, tsl]
                if tt >= LATE_T:
                    eng = nc.vector
                else:
                    eng = copy_engines[copy_rr[0] % len(copy_engines)]
                copy_rr[0] += 1
                if eng is nc.scalar:
                    nc.scalar.copy(out=dst, in_=ps_t)
                else:
                    eng.tensor_copy(out=dst, in_=ps_t)

        def phase_b_kq(c0, c1):
            tsl = slice(c0, c1)
            pkq = psB.tile([P, c1 - c0], f32, name="pkq", tag="bt")
            for ci in range(CT):
                nc.tensor.matmul(
                    pkq,
                    wkq_sb[:, ci, :],
                    xT[:, ci, tsl],
                    start=(ci == 0),
                    stop=(ci == CT - 1),
                )
            if c0 >= LATE_T * P:
                nc.vector.tensor_copy(out=kT[:, tsl], in_=pkq[0:H, :])
            else:
                nc.scalar.copy(out=kT[:, tsl], in_=pkq[0:H, :])
            # partition-shift copy 64-127 -> 0-63 (legal on DVE)
            nc.vector.tensor_copy(out=qT[:, tsl], in_=pkq[H:P, :])

        def phase_b_v(c0, c1):
            tsl = slice(c0, c1)
            pv = psB.tile([H, c1 - c0], f32, name="pv", tag="bt")
            for ci in range(CT):
                nc.tensor.matmul(
                    pv,
                    wv_sb[:, ci, :],
                    xT[:, ci, tsl],
                    start=(ci == 0),
                    stop=(ci == CT - 1),
                )
            if c0 >= LATE_T * P:
                nc.vector.tensor_copy(out=vT[:, tsl], in_=pv)
            else:
                nc.scalar.copy(out=vT[:, tsl], in_=pv)
            for c4 in range((c1 - c0) // P):
                st = c0 // P + c4
                pvt = psB.tile([P, H], tdt, name="pvt", tag="bt")
                nc.tensor.transpose(
                    pvt, vT[:, st * P : (st + 1) * P], idT[:H, :H]
                )
                nc.vector.tensor_copy(out=v1[:, st, 0:H], in_=pvt)

        def c_block(cb, diag_first=False, mask_dve=False):
            pe_pool, pe_tag = (psA, "pst") if (
                PE_POOL_LAST and cb >= 6
            ) else (psO, "po")
            """attention block cb: S-pair(g) / PV-pair(g) / finish closures.
            The diagonal pair (g=cb) is emitted FIRST so its mask chain is
            off the block's critical tail; PSUM start/stop flags follow
            emission order (accumulation order is irrelevant to the sum)."""
            tsl = slice(cb * BLK, (cb + 1) * BLK)
            state = {}
            order = [cb] + list(range(cb)) if diag_first else list(range(cb + 1))
            first_pair = order[0]
            last_pair = order[-1]

            def start():
                state["po"] = psO.tile([H1, BLK], f32, name="po", tag="po")
                state["pt"] = {}

            def s_pair(g):
                ps_s = psS.tile([P, 2, BLK], f32, name="ps_s")
                for j in range(2):
                    st = 2 * g + j
                    nc.tensor.matmul(
                        ps_s[:, j, :],
                        kT[:, st * P : (st + 1) * P],
                        qT[:, tsl],
                        start=True,
                        stop=True,
                    )
                ptile = pt_pool.tile([P, 2, BLK], bf16, name="ptile")
                state["pt"][g] = ptile
                if g < cb:
                    nc.scalar.activation(ptile, ps_s, Exp, scale=scale)
                else:
                    nc.scalar.activation(
                        ptile[:, 0, :], ps_s[:, 0, :], Exp, scale=scale
                    )
                    nc.scalar.activation(
                        ptile[:, 1, P:BLK], ps_s[:, 1, P:BLK], Exp, scale=scale
                    )
                    nc.gpsimd.memset(ptile[:, 1, 0:P], 0.0)
                    meng = nc.vector if mask_dve else nc.gpsimd
                    meng.tensor_mul(ptile[:, 0, 0:P], ptile[:, 0, 0:P], mask)
                    meng.tensor_mul(
                        ptile[:, 1, P:BLK], ptile[:, 1, P:BLK], mask
                    )

            def pv_pair(g):
                po = state["po"]
                ptile = state["pt"].pop(g)
                for j in range(2):
                    st = 2 * g + j
                    nc.tensor.matmul(
                        po,
                        v1[:, st, 0:H1],
                        ptile[:, j, :],
                        start=(g == first_pair and j == 0),
                        stop=(g == last_pair and j == 1),
                    )

            def finish():
                po = state["po"]
                oT = oT_pool.tile([H1, BLK], tdt, name="oT")
                for c4 in range(BLK // P):
                    csl = slice(c4 * P, (c4 + 1) * P)
                    odd = c4 % 2 == 1
                    if odd and EPI_PAR:
                        nc.scalar.copy(out=oT[:, csl], in_=po[:, csl])
                    else:
                        nc.vector.tensor_copy(out=oT[:, csl], in_=po[:, csl])
                    st = cb * (BLK // P) + c4
                    pe = pe_pool.tile([P, H1], tdt, name="pe", tag=pe_tag)
                    nc.tensor.transpose(pe, oT[:, csl], idT[:H1, :H1])
                    rec = rec_pool.tile([P, 1], f32, name="rec")
                    nc.vector.reciprocal(rec, pe[:, H:H1])
                    if odd and EPI_PAR:
                        nc.scalar.mul(out_sb[:, st, :], pe[:, 0:H], rec)
                    else:
                        nc.vector.tensor_scalar_mul(
                            out_sb[:, st, :], pe[:, 0:H], rec
                        )
                    nc.sync.dma_start(
                        out_d.rearrange("(o p) h -> p o h", p=P)[:, st, :],
                        out_sb[:, st, :],
                    )

            return start, s_pair, pv_pair, finish, order

        def phase_c_group(cbs, pv_offset, diag_first_last=False,
                          mask_dve=False, inject=None):
            """emit several C blocks with all their S-pairs zipped ahead of
            the PV-pairs (offset in pairs) so the exp stream saturates ACT."""
            blocks = {}
            seq = []
            for cb in cbs:
                df = diag_first_last and cb == cbs[-1]
                start, s_pair, pv_pair, finish, order = c_block(
                    cb, df, mask_dve
                )
                start()
                blocks[cb] = (s_pair, pv_pair, finish)
                seq.extend((cb, g) for g in order)
            done = 0
            inject = inject or {}
            for i, (cb, g) in enumerate(seq):
                blocks[cb][0](g)
                if i in inject:
                    inject[i]()
                if i >= pv_offset:
                    pcb, pg = seq[i - pv_offset]
                    blocks[pcb][1](pg)
                    done = i - pv_offset + 1
                    if done < len(seq) and seq[done][0] != pcb and (
                        done == 0 or seq[done - 1][0] == pcb
                    ):
                        blocks[pcb][2]()
            for i in range(done, len(seq)):
                pcb, pg = seq[i]
                blocks[pcb][1](pg)
                if i + 1 == len(seq) or seq[i + 1][0] != pcb:
                    blocks[pcb][2]()

        def phase_c(cb, diag_first=False, mask_dve=False):
            start, s_pair, pv_pair, finish, order = c_block(
                cb, diag_first, mask_dve
            )
            start()
            for i, g in enumerate(order):
                s_pair(g)
                if i >= PV_OFFSET:
                    pv_pair(order[i - PV_OFFSET])
            for i in range(max(0, len(order) - PV_OFFSET), len(order)):
                pv_pair(order[i])
            finish()

        def emit(tokens):
            for t in tokens.split():
                if t.startswith("A"):
                    phase_a(int(t[1:]))
                elif t.startswith("K"):
                    c0, c1 = t[1:].split("-")
                    phase_b_kq(int(c0), int(c1))
                elif t.startswith("V"):
                    c0, c1 = t[1:].split("-")
                    phase_b_v(int(c0), int(c1))
                elif t.startswith("C"):
                    cb = int(t[1:])
                    phase_c(cb, diag_first=(cb == 7 and DIAG_LAST7),
                            mask_dve=(cb >= 6))

        if LADDER == 10:
            phase_a(0)
            phase_a(1)
            phase_a(2)
            phase_a(3)
            phase_b_kq(0, 512)
            phase_c_group([0, 1], GRP_OFF,
                          inject={0: lambda: phase_b_v(0, 512)})
            phase_a(4)
            phase_a(5)
            phase_a(6)
            phase_a(7)
            phase_b_kq(512, 1024)
            phase_c_group([2, 3], GRP_OFF,
                          inject={1: lambda: phase_b_v(512, 1024)})
            phase_a(8)
            phase_a(9)
            phase_a(10)
            phase_a(11)
            phase_b_kq(1024, 1536)
            phase_c_group([4, 5], GRP_OFF,
                          inject={1: lambda: phase_b_v(1024, 1536)})
            phase_a(12)
            phase_a(13)
            phase_b_kq(1536, 1792)
            phase_c_group([6], GRP_OFF, mask_dve=True,
                          inject={1: lambda: phase_b_v(1536, 1792)})
            phase_a(14)
            phase_a(15)
            phase_b_kq(1792, 2048)
            phase_c_group([7], GRP_OFF, diag_first_last=DIAG_LAST7,
                          mask_dve=True,
                          inject={1: lambda: phase_b_v(1792, 2048)})
            return

        if LADDER == 9:
            phase_a(0)
            phase_a(1)
            phase_a(2)
            phase_a(3)
            phase_b_kq(0, 512)
            phase_b_v(0, 512)
            phase_c_group([0, 1], GRP_OFF)
            phase_a(4)
            phase_a(5)
            phase_a(6)
            phase_a(7)
            phase_b_kq(512, 1024)
            phase_b_v(512, 1024)
            phase_c_group([2, 3], GRP_OFF)
            phase_a(8)
            phase_a(9)
            phase_a(10)
            phase_a(11)
            phase_b_kq(1024, 1536)
            phase_b_v(1024, 1536)
            phase_c_group([4, 5], GRP_OFF)
            phase_a(12)
            phase_a(13)
            phase_b_kq(1536, 1792)
            phase_b_v(1536, 1792)
            phase_c(6, mask_dve=True)
            phase_a(14)
            phase_a(15)
            phase_b_kq(1792, 2048)
            phase_b_v(1792, 2048)
            phase_c(7, diag_first=DIAG_LAST7, mask_dve=True)
            return

        if LADDER == 8:
            phase_a(0)
            phase_a(1)
            phase_a(2)
            phase_a(3)
            phase_b_kq(0, 512)
            phase_a(4)
            phase_b_v(0, 512)
            phase_a(5)
            phase_c_group([0, 1], GRP_OFF)
            phase_a(6)
            phase_a(7)
            phase_b_kq(512, 1024)
            phase_a(8)
            phase_b_v(512, 1024)
            phase_a(9)
            phase_c_group([2, 3], GRP_OFF)
            phase_a(10)
            phase_a(11)
            phase_b_kq(1024, 1536)
            phase_a(12)
            phase_b_v(1024, 1536)
            phase_a(13)
            phase_c_group([4, 5], GRP_OFF)
            phase_a(14)
            phase_a(15)
            phase_b_kq(1536, 1792)
            phase_b_v(1536, 1792)
            phase_b_kq(1792, 2048)
            phase_b_v(1792, 2048)
            phase_c_group([6, 7], GRP_OFF, diag_first_last=DIAG_LAST7,
                          mask_dve=True)
            return

        LADDERS = {
            # best-known v2-style interleave with split last B block
            1: "A0 A1 A2 A3 K0-512 A4 V0-512 A5 C0 C1 A6 A7 "
               "K512-1024 A8 V512-1024 A9 C2 A10 C3 A11 "
               "K1024-1536 A12 V1024-1536 A13 C4 A14 C5 A15 "
               "K1536-1792 V1536-1792 C6 K1792-2048 V1792-2048 C7",
            # C-first everywhere
            2: "A0 A1 A2 A3 K0-512 V0-512 C0 C1 A4 A5 A6 A7 "
               "K512-1024 V512-1024 C2 C3 A8 A9 A10 A11 "
               "K1024-1536 V1024-1536 C4 C5 A12 A13 "
               "K1536-1792 V1536-1792 C6 A14 A15 K1792-2048 V1792-2048 C7",
            # hybrid: keep early interleave, pull late C blocks ahead of A14/A15
            3: "A0 A1 A2 A3 K0-512 A4 V0-512 A5 C0 C1 A6 A7 "
               "K512-1024 A8 V512-1024 A9 C2 A10 C3 A11 "
               "K1024-1536 A12 V1024-1536 A13 C4 C5 A14 A15 "
               "K1536-1792 V1536-1792 C6 K1792-2048 V1792-2048 C7",
            # hybrid + C4 before A13
            4: "A0 A1 A2 A3 K0-512 A4 V0-512 A5 C0 C1 A6 A7 "
               "K512-1024 A8 V512-1024 A9 C2 A10 C3 A11 "
               "K1024-1536 A12 V1024-1536 C4 A13 C5 A14 A15 "
               "K1536-1792 V1536-1792 C6 K1792-2048 V1792-2048 C7",
            # L1 + C2/C3 earlier relative to A9-A11
            5: "A0 A1 A2 A3 K0-512 A4 V0-512 A5 C0 C1 A6 A7 "
               "K512-1024 A8 V512-1024 C2 A9 C3 A10 A11 "
               "K1024-1536 A12 V1024-1536 A13 C4 A14 C5 A15 "
               "K1536-1792 V1536-1792 C6 K1792-2048 V1792-2048 C7",
            7: "A0 A1 A2 A3 K0-256 V0-256 C0 K256-512 V256-512 C1 A4 A5 "
               "K512-768 V512-768 C2 A6 A7 K768-1024 V768-1024 C3 A8 A9 "
               "K1024-1280 V1024-1280 C4 A10 A11 K1280-1536 V1280-1536 C5 "
               "A12 A13 K1536-1792 V1536-1792 C6 A14 A15 "
               "K1792-2048 V1792-2048 C7",
            # L5 + L4 combined
            6: "A0 A1 A2 A3 K0-512 A4 V0-512 A5 C0 C1 A6 A7 "
               "K512-1024 A8 V512-1024 C2 A9 C3 A10 A11 "
               "K1024-1536 A12 V1024-1536 C4 A13 C5 A14 A15 "
               "K1536-1792 V1536-1792 C6 K1792-2048 V1792-2048 C7",
        }
        emit(LADDERS[LADDER])


_NC_CACHE = {}


def build_nc():
    if "nc" in _NC_CACHE:
        return _NC_CACHE["nc"]
    _patch_drain_split()
    f32r = mybir.dt.float32r
    f32 = mybir.dt.float32
    nc = bass.Bass(
        "TRN2", target_bir_lowering=False, debug=False, num_devices=N_CORES
    )
    x_d = nc.dram_tensor("x", [T, C], f32r, kind="ExternalInput").ap()
    wk_d = nc.dram_tensor("Wk", [C, H], f32r, kind="ExternalInput").ap()
    wq_d = nc.dram_tensor("Wq", [C, H], f32r, kind="ExternalInput").ap()
    wv_d = nc.dram_tensor("Wv", [C, H], f32r, kind="ExternalInput").ap()
    out_d = nc.dram_tensor("out", [T, H], f32, kind="ExternalOutput").ap()
    with tile.TileContext(nc) as tc:
        _emit(tc, out_d, x_d, wk_d, wq_d, wv_d)
    _NC_CACHE["nc"] = nc
    return nc


def kernel(x, Wk, Wq, Wv, **run_kwargs):
    """Full-input entry point: shard over batch, run on cores 0-7, gather."""
    x = np.ascontiguousarray(np.asarray(x), dtype=np.float32)
    Wk = np.ascontiguousarray(np.asarray(Wk), dtype=np.float32)
    Wq = np.ascontiguousarray(np.asarray(Wq), dtype=np.float32)
    Wv = np.ascontiguousarray(np.asarray(Wv), dtype=np.float32)
    assert x.shape == (B, T, C), x.shape

    nc = build_nc()
    in_maps = [
        {"x": np.ascontiguousarray(x[b]), "Wk": Wk, "Wq": Wq, "Wv": Wv}
        for b in range(B)
    ]
    res = bass_utils.run_bass_kernel_spmd(
        nc, in_maps, core_ids=list(range(N_CORES)), **run_kwargs
    )
    out = np.stack([res.results[b]["out"] for b in range(B)], axis=0)
    if run_kwargs:
        kernel.last_results = res
    return out.astype(np.float32)


# revision 45
# speedup vs baseline: 1.0344x; 1.0061x over previous
"""nn_Head single-head causal attention on 8 TRN2 NeuronCores.

Full inputs: x [8, 2048, 1024] f32, Wk/Wq/Wv [1024, 64] f32.
Full output: [8, 2048, 64] f32 = softmax(causal(q k^T * C^-0.5)) @ v per batch.

Sharding: data-parallel over batch B=8 -> one batch element per core;
weights replicated. No collectives.

Per-core kernel (Bass/Tile, f32r matmuls + bf16 probability/value stage):
  A) x t-tiles DMA in (first two tiles split in halves to cut first-arrival
     latency); PE-transpose to xT [c-part, t] (TensorE contracts over the
     partition dim, fp32 cannot DMA-transpose).  All PE transposes use a
     bf16 identity as the moving operand: the moving dtype sets the
     cycles/row (bf16 1.0 vs f32r 1.5 / f32 2.0), while values stay exact.
  B) QKV per 512-col block: kT/qT/vT [h(64), t] via lhsT=W [c,64], rhs=xT;
     v transposed back to natural v1 [s-part, t-tile, H+1] bf16 with a ones
     column at H that makes the PV matmul also produce the softmax
     denominator.  W is DMAed in half-tensors interleaved between x tiles.
  C) attention per 256-col block: S^T tile = kT_slice^T@qT -> PSUM
     [s 128, t 256]; exp on ScalarE with scale=C^-0.5 folded in (scores are
     O(1): no max-subtraction needed, mathematically identical softmax);
     causality via memset of fully-masked column ranges + a 0/1
     upper-triangular mask mul on diagonal tiles; PV: po[h|denom, t] +=
     v1_slice^T @ P^T accumulated over s-tiles; po -> SBUF (f32r),
     PE-transpose to [t-part, H+1], multiply by per-partition reciprocal of
     the denominator column, streaming output DMAs.
  Emission interleaves B/C blocks between A tiles in data-arrival order so
  the PE always has ready work (each engine has a 4-deep scoreboard that
  lets ready instructions pass blocked ones).
"""

from contextlib import ExitStack

import numpy as np

import concourse.bass as bass
import concourse.mybir as mybir
import concourse.tile as tile
from concourse import bass_utils
from concourse.masks import make_identity

B, T, C, H = 8, 2048, 1024, 64
N_CORES = 8
P = 128

# ---- tuning knobs (swept via TimelineSim) ----
IDENT_B = True  # bf16 identity for all PE transposes
ACT_CLEAN = True  # keep ScalarE free for exp: xT/kT copies on Pool/DVE
PSA1_PSS3 = False  # psA bufs 1 / psS bufs 3 (vs 2/2)
W_EARLY = False  # W DMA halves at tt1/2/3 (vs tt3/4/5/6)
OUT_TILE = True  # per-tile output DMAs (vs per-block)
ZIP67 = False  # zip the last two attention blocks pair-by-pair
BLKB_W = 512  # qkv block width
SCHED = "v2"  # "v2" ladder or "weave"
PV_OFFSET = 2  # emit PV pairs this many pairs behind S pairs
DIAG_FIRST = False  # emit the diagonal pair first within each C block
PSB1_PSS3 = False  # psB bufs 1, psS bufs 3
LADDER = 8  # emission order variant
DIAG_LAST7 = True  # diagonal pair first in the final block
PT_BUFS = 16  # ptile pool depth
GRP_OFF = 5  # PV offset within a grouped C emission
XT_RATIO = 1  # xT copy engine mix: 0=3:1 DVE:ACT, 1=1:1, 2=all DVE
W_SLOT = 0  # which x-tile slots carry the W DMA halves
PE_POOL_LAST = False  # late C blocks take pe tiles from the idle psA pool
X_SPLIT = 16  # how many leading x tiles DMA in half-tiles
LATE_T = 9  # from this x-tile / col on, copies avoid ScalarE
EPI_PAR = False  # parallel DVE/ACT epilogue chunk chains
OUT_POOL = False  # issue output DMAs from the Pool engine (SWDGE)
EPI_PAR7 = False  # parallel epilogue chains for the final block only


def _patch_drain_split():
    """This walrus build accepts only one sem wait per instruction ("Too many
    sync wait commands" in setupSyncWait otherwise). Hoist extra waits onto
    same-engine NOPs ahead of the instruction (engine streams dispatch
    in-order, so the blocking semantics are identical), and split the
    TileContext tail drain the same way."""
    if getattr(tile.TileContext, "_drain_split_patched", False):
        return
    from concourse.tile import ScopedClock

    _orig_add = tile.TileContext._add_instruction

    def _patched_add(self, inst):
        si = getattr(inst, "sync_info", None)
        if si is not None and si.on_wait and len(si.on_wait) > 1:
            waits = list(si.on_wait)
            for i, w in enumerate(waits[:-1]):
                nop = mybir.InstNoOp(
                    name=f"{inst.name}-ws{i}",
                    sync_info=mybir.SyncInfo(on_wait=[w], on_update=[]),
                    bass_nofuse=True,
                    engine=inst.engine,
                )
                _orig_add(self, nop)
            si.on_wait = waits[-1:]
            inst.sync_info = si
        _orig_add(self, inst)

    tile.TileContext._add_instruction = _patched_add

    def _patched_dab(self, tick_clock, wait_clock):
        nc = self.nc
        drain_inst = nc.sync.drain()
        wait_clock.add_sem_waits(
            drain_inst.ins, ScopedClock({None: tick_clock.global_clock})
        )
        si = drain_inst.ins.sync_info
        if si is not None and si.on_wait and len(si.on_wait) > 1:
            waits = list(si.on_wait)
            si.on_wait = waits[:1]
            drain_inst.ins.sync_info = si
            for w in waits[1:]:
                d2 = nc.sync.drain()
                d2.ins.sync_info = mybir.SyncInfo(on_wait=[w], on_update=[])
        nc.all_engine_barrier()
        popped = nc._tile_sem_poison_stack.pop()
        assert popped is self._sem_poison
        nc.clear_and_free_semaphores(list(self.sems.allocated().values()))
        nc.all_engine_barrier()

    tile.TileContext._drain_and_barrier = _patched_dab
    tile.TileContext._drain_split_patched = True


def _emit(tc, out_d, x_d, wk_d, wq_d, wv_d):
    nc = tc.nc
    f32r = mybir.dt.float32r
    f32 = mybir.dt.float32
    bf16 = mybir.dt.bfloat16
    Exp = mybir.ActivationFunctionType.Exp

    CT = C // P  # 8 c-tiles
    TT = T // P  # 16 t-tiles
    BLKB = BLKB_W  # qkv block width
    BLK = 256  # attention block width
    H1 = H + 1
    scale = float(C) ** -0.5
    tdt = bf16  # vT/oT dtype: bf16 so their transposes use the bf16 identity

    with ExitStack() as ctx:
        const = ctx.enter_context(tc.tile_pool(name="const", bufs=1))
        persist = ctx.enter_context(tc.tile_pool(name="persist", bufs=1))
        xa_pool = ctx.enter_context(tc.tile_pool(name="xa", bufs=TT))
        pt_pool = ctx.enter_context(tc.tile_pool(name="ptp", bufs=PT_BUFS))
        oT_pool = ctx.enter_context(tc.tile_pool(name="otp", bufs=OT_BUFS))
        rec_pool = ctx.enter_context(tc.tile_pool(name="recp", bufs=2))
        # PSUM: 8 banks total so all phases can overlap.
        psA = ctx.enter_context(
            tc.tile_pool(name="psA", bufs=1 if PSA1_PSS3 else 2, space="PSUM")
        )
        psB = ctx.enter_context(tc.tile_pool(name="psB", bufs=1 if PSB1_PSS3 else 2, space="PSUM"))
        psS = ctx.enter_context(
            tc.tile_pool(
                name="psS",
                bufs=3 if (PSA1_PSS3 or PSB1_PSS3) else 2,
                space="PSUM",
            )
        )
        psO = ctx.enter_context(tc.tile_pool(name="psO", bufs=PSO_BUFS, space="PSUM"))

        # identity: build in f32 (memset on f32r is invalid ISA in this
        # walrus); f32r and bf16 copies for dtype-matched transposes
        ident = const.tile([P, P], f32, name="ident")
        make_identity(nc, ident)
        identr = const.tile([P, P], f32r, name="identr")
        nc.vector.tensor_copy(out=identr, in_=ident)
        identb = const.tile([P, P], bf16, name="identb")
        nc.vector.tensor_copy(out=identb, in_=ident)
        idT = identb if IDENT_B else identr
        # 0/1 mask: mask[s, t] = 1 iff s <= t (keep causal entries)
        mask = const.tile([P, P], bf16, name="mask")
        nc.vector.memset(mask, 1.0)
        nc.gpsimd.affine_select(
            out=mask,
            in_=mask,
            compare_op=mybir.AluOpType.is_ge,
            fill=0.0,
            base=0,
            pattern=[[1, P]],
            channel_multiplier=-1,
        )

        # [Wk | Wq] packed: one M=128 matmul produces k on partitions 0-63
        # and q on 64-127
        wkq_sb = const.tile([P, CT, 2 * H], f32r, name="wkq_sb")
        wv_sb = const.tile([P, CT, H], f32r, name="wv_sb")

        xT = persist.tile([P, CT, T], f32r, name="xT")
        kT = persist.tile([H, T], f32r, name="kT")
        qT = persist.tile([H, T], f32r, name="qT")
        vT = persist.tile([H, T], tdt, name="vT")
        v1 = persist.tile([P, TT, H1], bf16, name="v1")
        out_sb = persist.tile([P, TT, H], f32, name="out_sb")

        nc.vector.memset(v1[:, :, H : H + 1], 1.0)

        # ---- DMA emission (SP stream order == DMA device service order).
        wk_r = wk_d.rearrange("(o p) h -> p o h", p=P)
        wq_r = wq_d.rearrange("(o p) h -> p o h", p=P)
        wv_r = wv_d.rearrange("(o p) h -> p o h", p=P)
        wslots = {
            0: {3: 0, 4: 1, 5: 2, 6: 3},
            1: {2: 0, 3: 1, 4: 2, 5: 3},
            2: {1: 0, 2: 1, 3: 2, 4: 3},
        }[W_SLOT]
        xa_tiles = []
        for tt in range(TT):
            tsl = slice(tt * P, (tt + 1) * P)
            xa = xa_pool.tile([P, C], f32r, name="xa")
            xa_tiles.append(xa)
            if tt < X_SPLIT:
                nc.sync.dma_start(xa[:, 0 : C // 2], x_d[tsl, 0 : C // 2])
                nc.sync.dma_start(xa[:, C // 2 : C], x_d[tsl, C // 2 : C])
            else:
                nc.sync.dma_start(xa, x_d[tsl, :])
            w = wslots.get(tt)
            if False:
                if w == 0:
                    nc.sync.dma_start(wkq_sb[:, :, 0:H], wk_r)
                elif w == 1:
                    nc.sync.dma_start(wkq_sb[:, :, H : 2 * H], wq_r)
                elif w == 2:
                    nc.sync.dma_start(wv_sb, wv_r)
            elif w is not None:
                if w == 0:
                    nc.sync.dma_start(wkq_sb[:, 0:4, 0:H], wk_r[:, 0:4, :])
                    nc.sync.dma_start(
                        wkq_sb[:, 0:4, H : 2 * H], wq_r[:, 0:4, :]
                    )
                elif w == 1:
                    nc.sync.dma_start(wkq_sb[:, 4:CT, 0:H], wk_r[:, 4:CT, :])
                    nc.sync.dma_start(
                        wkq_sb[:, 4:CT, H : 2 * H], wq_r[:, 4:CT, :]
                    )
                elif w == 2:
                    nc.sync.dma_start(wv_sb[:, 0:4, :], wv_r[:, 0:4, :])
                elif w == 3:
                    nc.sync.dma_start(wv_sb[:, 4:CT, :], wv_r[:, 4:CT, :])

        # ---- compute emission helpers
        copy_rr = [0]
        if XT_RATIO == 0:
            copy_engines = [nc.vector, nc.vector, nc.scalar, nc.vector]
        elif XT_RATIO == 1:
            copy_engines = [nc.vector, nc.scalar]
        elif XT_RATIO == 3:
            copy_engines = [nc.vector, nc.scalar, nc.vector]
        elif XT_RATIO == 4:
            copy_engines = [nc.scalar, nc.vector, nc.scalar]
        else:
            copy_engines = [nc.vector]

        def phase_a(tt):
            """transpose x tile tt into xT."""
            tsl = slice(tt * P, (tt + 1) * P)
            xa = xa_tiles[tt]
            for cg in range(2):
                ps_t = psA.tile([P, 4, P], f32r, name="ps_t", tag="pst")
                for j in range(4):
                    ci = cg * 4 + j
                    nc.tensor.transpose(
                        ps_t[:, j, :], xa[:, ci * P : (ci + 1) * P], identr
                    )
                dst = xT[:, cg * 4 : cg * 4 + 4, tsl]
                if tt >= LATE_T:
                    eng = nc.vector
                else:
                    eng = copy_engines[copy_rr[0] % len(copy_engines)]
                copy_rr[0] += 1
                if eng is nc.scalar:
                    nc.scalar.copy(out=dst, in_=ps_t)
                else:
                    eng.tensor_copy(out=dst, in_=ps_t)

        def phase_b_kq(c0, c1):
            tsl = slice(c0, c1)
            pkq = psB.tile([P, c1 - c0], f32, name="pkq", tag="bt")
            for ci in range(CT):
                nc.tensor.matmul(
                    pkq,
                    wkq_sb[:, ci, :],
                    xT[:, ci, tsl],
                    start=(ci == 0),
                    stop=(ci == CT - 1),
                )
            if c0 >= LATE_T * P:
                nc.vector.tensor_copy(out=kT[:, tsl], in_=pkq[0:H, :])
            else:
                nc.scalar.copy(out=kT[:, tsl], in_=pkq[0:H, :])
            # partition-shift copy 64-127 -> 0-63 (legal on DVE)
            nc.vector.tensor_copy(out=qT[:, tsl], in_=pkq[H:P, :])

        def phase_b_v(c0, c1):
            tsl = slice(c0, c1)
            pv = psB.tile([H, c1 - c0], f32, name="pv", tag="bt")
            for ci in range(CT):
                nc.tensor.matmul(
                    pv,
                    wv_sb[:, ci, :],
                    xT[:, ci, tsl],
                    start=(ci == 0),
                    stop=(ci == CT - 1),
                )
            if c0 >= LATE_T * P:
                nc.vector.tensor_copy(out=vT[:, tsl], in_=pv)
            else:
                nc.scalar.copy(out=vT[:, tsl], in_=pv)
            for c4 in range((c1 - c0) // P):
                st = c0 // P + c4
                pvt = psB.tile([P, H], tdt, name="pvt", tag="bt")
                nc.tensor.transpose(
                    pvt, vT[:, st * P : (st + 1) * P], idT[:H, :H]
                )
                nc.vector.tensor_copy(out=v1[:, st, 0:H], in_=pvt)

        def c_block(cb, diag_first=False, mask_dve=False):
            pe_pool, pe_tag = (psA, "pst") if (
                PE_POOL_LAST and cb >= 6
            ) else (psO, "po")
            """attention block cb: S-pair(g) / PV-pair(g) / finish closures.
            The diagonal pair (g=cb) is emitted FIRST so its mask chain is
            off the block's critical tail; PSUM start/stop flags follow
            emission order (accumulation order is irrelevant to the sum)."""
            tsl = slice(cb * BLK, (cb + 1) * BLK)
            state = {}
            if diag_first == 1:
                order = [cb] + list(range(cb))
            elif diag_first == 2 and cb >= 1:
                order = list(range(cb - 1)) + [cb, cb - 1]
            else:
                order = list(range(cb + 1))
            first_pair = order[0]
            last_pair = order[-1]

            def start():
                state["po"] = psO.tile([H1, BLK], f32, name="po", tag="po")
                state["pt"] = {}

            def s_pair(g):
                ps_s = psS.tile([P, 2, BLK], f32, name="ps_s")
                for j in range(2):
                    st = 2 * g + j
                    nc.tensor.matmul(
                        ps_s[:, j, :],
                        kT[:, st * P : (st + 1) * P],
                        qT[:, tsl],
                        start=True,
                        stop=True,
                    )
                ptile = pt_pool.tile([P, 2, BLK], bf16, name="ptile")
                state["pt"][g] = ptile
                if g < cb:
                    nc.scalar.activation(ptile, ps_s, Exp, scale=scale)
                else:
                    nc.scalar.activation(
                        ptile[:, 0, :], ps_s[:, 0, :], Exp, scale=scale
                    )
                    nc.scalar.activation(
                        ptile[:, 1, P:BLK], ps_s[:, 1, P:BLK], Exp, scale=scale
                    )
                    if mask_dve and MEMSET_DVE:
                        nc.vector.memset(ptile[:, 1, 0:P], 0.0)
                    else:
                        nc.gpsimd.memset(ptile[:, 1, 0:P], 0.0)
                    meng = nc.vector if mask_dve else nc.gpsimd
                    meng.tensor_mul(ptile[:, 0, 0:P], ptile[:, 0, 0:P], mask)
                    meng.tensor_mul(
                        ptile[:, 1, P:BLK], ptile[:, 1, P:BLK], mask
                    )

            def pv_pair(g):
                po = state["po"]
                ptile = state["pt"].pop(g)
                for j in range(2):
                    st = 2 * g + j
                    nc.tensor.matmul(
                        po,
                        v1[:, st, 0:H1],
                        ptile[:, j, :],
                        start=(g == first_pair and j == 0),
                        stop=(g == last_pair and j == 1),
                    )

            def finish():
                po = state["po"]
                oT = oT_pool.tile([H1, BLK], tdt, name="oT")
                for c4 in range(BLK // P):
                    csl = slice(c4 * P, (c4 + 1) * P)
                    odd = c4 % 2 == 1
                    par = EPI_PAR or (EPI_PAR7 and cb == 7)
                    if odd and par:
                        nc.scalar.copy(out=oT[:, csl], in_=po[:, csl])
                    else:
                        nc.vector.tensor_copy(out=oT[:, csl], in_=po[:, csl])
                    st = cb * (BLK // P) + c4
                    pe = pe_pool.tile([P, H1], tdt, name="pe", tag=pe_tag)
                    nc.tensor.transpose(pe, oT[:, csl], idT[:H1, :H1])
                    rec = rec_pool.tile([P, 1], f32, name="rec")
                    nc.vector.reciprocal(rec, pe[:, H:H1])
                    if odd and par:
                        nc.scalar.mul(out_sb[:, st, :], pe[:, 0:H], rec)
                    else:
                        nc.vector.tensor_scalar_mul(
                            out_sb[:, st, :], pe[:, 0:H], rec
                        )
                    out_eng = nc.gpsimd if OUT_POOL else nc.sync
                    out_eng.dma_start(
                        out_d.rearrange("(o p) h -> p o h", p=P)[:, st, :],
                        out_sb[:, st, :],
                    )

            return start, s_pair, pv_pair, finish, order

        def phase_c_group(cbs, pv_offset, diag_first_last=False,
                          mask_dve=False, inject=None):
            """emit several C blocks with all their S-pairs zipped ahead of
            the PV-pairs (offset in pairs) so the exp stream saturates ACT."""
            blocks = {}
            seq = []
            for cb in cbs:
                df = diag_first_last if cb == cbs[-1] else 0
                start, s_pair, pv_pair, finish, order = c_block(
                    cb, df, mask_dve
                )
                start()
                blocks[cb] = (s_pair, pv_pair, finish)
                seq.extend((cb, g) for g in order)
            done = 0
            inject = inject or {}
            for i, (cb, g) in enumerate(seq):
                blocks[cb][0](g)
                if i in inject:
                    inject[i]()
                if i >= pv_offset:
                    pcb, pg = seq[i - pv_offset]
                    blocks[pcb][1](pg)
                    done = i - pv_offset + 1
                    if done < len(seq) and seq[done][0] != pcb and (
                        done == 0 or seq[done - 1][0] == pcb
                    ):
                        blocks[pcb][2]()
            for i in range(done, len(seq)):
                pcb, pg = seq[i]
                blocks[pcb][1](pg)
                if i + 1 == len(seq) or seq[i + 1][0] != pcb:
                    blocks[pcb][2]()

        def phase_c(cb, diag_first=False, mask_dve=False):
            start, s_pair, pv_pair, finish, order = c_block(
                cb, diag_first, mask_dve
            )
            start()
            for i, g in enumerate(order):
                s_pair(g)
                if i >= PV_OFFSET:
                    pv_pair(order[i - PV_OFFSET])
            for i in range(max(0, len(order) - PV_OFFSET), len(order)):
                pv_pair(order[i])
            finish()

        def emit(tokens):
            for t in tokens.split():
                if t.startswith("A"):
                    phase_a(int(t[1:]))
                elif t.startswith("K"):
                    c0, c1 = t[1:].split("-")
                    phase_b_kq(int(c0), int(c1))
                elif t.startswith("V"):
                    c0, c1 = t[1:].split("-")
                    phase_b_v(int(c0), int(c1))
                elif t.startswith("C"):
                    cb = int(t[1:])
                    phase_c(cb, diag_first=(cb == 7 and DIAG_LAST7),
                            mask_dve=(cb >= 6))

        if LADDER == 10:
            phase_a(0)
            phase_a(1)
            phase_a(2)
            phase_a(3)
            phase_b_kq(0, 512)
            phase_c_group([0, 1], GRP_OFF,
                          inject={0: lambda: phase_b_v(0, 512)})
            phase_a(4)
            phase_a(5)
            phase_a(6)
            phase_a(7)
            phase_b_kq(512, 1024)
            phase_c_group([2, 3], GRP_OFF,
                          inject={1: lambda: phase_b_v(512, 1024)})
            phase_a(8)
            phase_a(9)
            phase_a(10)
            phase_a(11)
            phase_b_kq(1024, 1536)
            phase_c_group([4, 5], GRP_OFF,
                          inject={1: lambda: phase_b_v(1024, 1536)})
            phase_a(12)
            phase_a(13)
            phase_b_kq(1536, 1792)
            phase_c_group([6], GRP_OFF, mask_dve=True,
                          inject={1: lambda: phase_b_v(1536, 1792)})
            phase_a(14)
            phase_a(15)
            phase_b_kq(1792, 2048)
            phase_c_group([7], GRP_OFF, diag_first_last=DIAG_LAST7,
                          mask_dve=True,
                          inject={1: lambda: phase_b_v(1792, 2048)})
            return

        if LADDER == 11:
            phase_a(0)
            phase_a(1)
            phase_a(2)
            phase_a(3)
            phase_b_kq(0, 512)
            phase_a(4)
            phase_b_v(0, 512)
            phase_a(5)
            phase_a(6)
            phase_a(7)
            phase_b_kq(512, 1024)
            phase_a(8)
            phase_b_v(512, 1024)
            phase_c_group([0, 1, 2, 3], GRP_OFF)
            phase_a(9)
            phase_a(10)
            phase_a(11)
            phase_b_kq(1024, 1536)
            phase_a(12)
            phase_b_v(1024, 1536)
            phase_a(13)
            phase_c_group([4, 5], GRP_OFF, mask_dve=(4 >= MASK_DVE_FROM))
            phase_a(14)
            phase_a(15)
            phase_b_kq(1536, 1792)
            phase_b_v(1536, 1792)
            phase_b_kq(1792, 2048)
            phase_b_v(1792, 2048)
            phase_c_group([6, 7], GRP_OFF, diag_first_last=DIAG_LAST7,
                          mask_dve=True)
            return

        if LADDER == 9:
            phase_a(0)
            phase_a(1)
            phase_a(2)
            phase_a(3)
            phase_b_kq(0, 512)
            phase_b_v(0, 512)
            phase_c_group([0, 1], GRP_OFF, mask_dve=(0 >= MASK_DVE_FROM))
            phase_a(4)
            phase_a(5)
            phase_a(6)
            phase_a(7)
            phase_b_kq(512, 1024)
            phase_b_v(512, 1024)
            phase_c_group([2, 3], GRP_OFF, mask_dve=(2 >= MASK_DVE_FROM))
            phase_a(8)
            phase_a(9)
            phase_a(10)
            phase_a(11)
            phase_b_kq(1024, 1536)
            phase_b_v(1024, 1536)
            phase_c_group([4, 5], GRP_OFF)
            phase_a(12)
            phase_a(13)
            phase_b_kq(1536, 1792)
            phase_b_v(1536, 1792)
            phase_c(6, mask_dve=True)
            phase_a(14)
            phase_a(15)
            phase_b_kq(1792, 2048)
            phase_b_v(1792, 2048)
            phase_c(7, diag_first=DIAG_LAST7, mask_dve=True)
            return

        if LADDER == 8:
            phase_a(0)
            phase_a(1)
            phase_a(2)
            phase_a(3)
            phase_b_kq(0, 512)
            phase_a(4)
            phase_b_v(0, 512)
            phase_a(5)
            phase_c_group([0, 1], GRP_OFF, mask_dve=(0 >= MASK_DVE_FROM))
            phase_a(6)
            phase_a(7)
            phase_b_kq(512, 1024)
            phase_a(8)
            phase_b_v(512, 1024)
            phase_a(9)
            phase_c_group([2, 3], GRP_OFF, mask_dve=(2 >= MASK_DVE_FROM))
            phase_a(10)
            phase_a(11)
            phase_b_kq(1024, 1536)
            phase_a(12)
            phase_b_v(1024, 1536)
            phase_a(13)
            phase_c_group([4, 5], GRP_OFF, mask_dve=(4 >= MASK_DVE_FROM))
            phase_a(14)
            phase_a(15)
            phase_b_kq(1536, 1792)
            phase_b_v(1536, 1792)
            phase_b_kq(1792, 2048)
            phase_b_v(1792, 2048)
            phase_c_group([6, 7], GRP_OFF, diag_first_last=DIAG_LAST7,
                          mask_dve=True)
            return

        LADDERS = {
            # best-known v2-style interleave with split last B block
            1: "A0 A1 A2 A3 K0-512 A4 V0-512 A5 C0 C1 A6 A7 "
               "K512-1024 A8 V512-1024 A9 C2 A10 C3 A11 "
               "K1024-1536 A12 V1024-1536 A13 C4 A14 C5 A15 "
               "K1536-1792 V1536-1792 C6 K1792-2048 V1792-2048 C7",
            # C-first everywhere
            2: "A0 A1 A2 A3 K0-512 V0-512 C0 C1 A4 A5 A6 A7 "
               "K512-1024 V512-1024 C2 C3 A8 A9 A10 A11 "
               "K1024-1536 V1024-1536 C4 C5 A12 A13 "
               "K1536-1792 V1536-1792 C6 A14 A15 K1792-2048 V1792-2048 C7",
            # hybrid: keep early interleave, pull late C blocks ahead of A14/A15
            3: "A0 A1 A2 A3 K0-512 A4 V0-512 A5 C0 C1 A6 A7 "
               "K512-1024 A8 V512-1024 A9 C2 A10 C3 A11 "
               "K1024-1536 A12 V1024-1536 A13 C4 C5 A14 A15 "
               "K1536-1792 V1536-1792 C6 K1792-2048 V1792-2048 C7",
            # hybrid + C4 before A13
            4: "A0 A1 A2 A3 K0-512 A4 V0-512 A5 C0 C1 A6 A7 "
               "K512-1024 A8 V512-1024 A9 C2 A10 C3 A11 "
               "K1024-1536 A12 V1024-1536 C4 A13 C5 A14 A15 "
               "K1536-1792 V1536-1792 C6 K1792-2048 V1792-2048 C7",
            # L1 + C2/C3 earlier relative to A9-A11
            5: "A0 A1 A2 A3 K0-512 A4 V0-512 A5 C0 C1 A6 A7 "
               "K512-1024 A8 V512-1024 C2 A9 C3 A10 A11 "
               "K1024-1536 A12 V1024-1536 A13 C4 A14 C5 A15 "
               "K1536-1792 V1536-1792 C6 K1792-2048 V1792-2048 C7",
            7: "A0 A1 A2 A3 K0-256 V0-256 C0 K256-512 V256-512 C1 A4 A5 "
               "K512-768 V512-768 C2 A6 A7 K768-1024 V768-1024 C3 A8 A9 "
               "K1024-1280 V1024-1280 C4 A10 A11 K1280-1536 V1280-1536 C5 "
               "A12 A13 K1536-1792 V1536-1792 C6 A14 A15 "
               "K1792-2048 V1792-2048 C7",
            # L5 + L4 combined
            6: "A0 A1 A2 A3 K0-512 A4 V0-512 A5 C0 C1 A6 A7 "
               "K512-1024 A8 V512-1024 C2 A9 C3 A10 A11 "
               "K1024-1536 A12 V1024-1536 C4 A13 C5 A14 A15 "
               "K1536-1792 V1536-1792 C6 K1792-2048 V1792-2048 C7",
        }
        emit(LADDERS[LADDER])


_NC_CACHE = {}


def build_nc():
    if "nc" in _NC_CACHE:
        return _NC_CACHE["nc"]
    _patch_drain_split()
    f32r = mybir.dt.float32r
    f32 = mybir.dt.float32
    nc = bass.Bass(
        "TRN2", target_bir_lowering=False, debug=False, num_devices=N_CORES
    )
    x_d = nc.dram_tensor("x", [T, C], f32r, kind="ExternalInput").ap()
    wk_d = nc.dram_tensor("Wk", [C, H], f32r, kind="ExternalInput").ap()
    wq_d = nc.dram_tensor("Wq", [C, H], f32r, kind="ExternalInput").ap()
    wv_d = nc.dram_tensor("Wv", [C, H], f32r, kind="ExternalInput").ap()
    out_d = nc.dram_tensor("out", [T, H], f32, kind="ExternalOutput").ap()
    with tile.TileContext(nc) as tc:
        _emit(tc, out_d, x_d, wk_d, wq_d, wv_d)
    _NC_CACHE["nc"] = nc
    return nc


def kernel(x, Wk, Wq, Wv, **run_kwargs):
    """Full-input entry point: shard over batch, run on cores 0-7, gather."""
    x = np.ascontiguousarray(np.asarray(x), dtype=np.float32)
    Wk = np.ascontiguousarray(np.asarray(Wk), dtype=np.float32)
    Wq = np.ascontiguousarray(np.asarray(Wq), dtype=np.float32)
    Wv = np.ascontiguousarray(np.asarray(Wv), dtype=np.float32)
    assert x.shape == (B, T, C), x.shape

    nc = build_nc()
    in_maps = [
        {"x": np.ascontiguousarray(x[b]), "Wk": Wk, "Wq": Wq, "Wv": Wv}
        for b in range(B)
    ]
    res = bass_utils.run_bass_kernel_spmd(
        nc, in_maps, core_ids=list(range(N_CORES)), **run_kwargs
    )
    out = np.stack([res.results[b]["out"] for b in range(B)], axis=0)
    if run_kwargs:
        kernel.last_results = res
    return out.astype(np.float32)
